# revision 52
# baseline (speedup 1.0000x reference)
"""TRN2 Bass kernel for nn_Attention_73839077752929.

Computes (matching the reference, which has the source bug k = v = q):
    q = x @ Wq^T + bq          (only the q-slice of Wqkv is ever used)
    a = softmax(causal(q q^T / 8)) @ q      per head
    y = a @ Wproj^T + bproj

Sharding: 8 cores = 4 batches x 2 head-groups (6 heads each).
Each core computes a partial projection output for its batch; the host
sums the two partials per batch and adds the projection bias.

On-core scheme (transposed-probability formulation, bf16 attention
operands, f32 PSUM accumulation):
    qT[d,t]   = wqT^T @ xT (+bias)  -> bf16   [head pairs on 128 partitions]
    V_i       = PE-transpose of qT (bf16), ones column pre-set
    PT        = exp((S^T)/8) per k-block PAIR (wide activations,
                multiplicative transposed-causal mask on diag blocks)
    OT'[d|1,q] = sum_i V_i^T @ PT_i    (extra row = softmax denominators)
    aT       *= bcast(1/denominators)   (deferred normalization)
    y[t,o]    = aT^T @ wpT
"""

import os

import numpy as np

N_CORES = 8
NB, NS, NF = 4, 2048, 768
N_HEADS_TOTAL = 12
HD = 64
NH = 6  # heads per core
DL = NH * HD  # 384 local dims
NPAIR = NH // 2  # 3 head pairs (128 partitions each)
NKB = NS // 128  # 16 k-blocks
NJC = NS // 512  # 4 q-chunks
NFC = NF // 128  # 6 feature chunks

_COMPILED = {}


def _build():
    import concourse.bacc as bacc
    import concourse.bass as bass
    import concourse.mybir as mybir
    import concourse.tile as tile
    from concourse.masks import make_identity

    F32 = mybir.dt.float32
    F32R = mybir.dt.float32r
    BF16 = mybir.dt.bfloat16
    F8 = mybir.dt.float8e4

    nc = bacc.Bacc(trn_type="TRN2", target_bir_lowering=False)

    xT_d = nc.dram_tensor("xT", [NF, NS], F32, kind="ExternalInput").ap()
    wqT_d = nc.dram_tensor("wqT", [NF, DL], F32, kind="ExternalInput").ap()
    bq_d = nc.dram_tensor("bq", [NPAIR, 128], F32, kind="ExternalInput").ap()
    wpT_d = nc.dram_tensor("wpT", [DL, NF], BF16, kind="ExternalInput").ap()
    y_d = nc.dram_tensor("y", [NS, NF], F32, kind="ExternalOutput").ap()

    with tile.TileContext(nc) as tc:
        with (
            tc.tile_pool(name="const", bufs=1) as constp,
            tc.tile_pool(name="w", bufs=1) as wp,
            tc.tile_pool(name="big", bufs=1) as bigp,
            tc.tile_pool(name="pt", bufs=7) as ptp,
            tc.tile_pool(name="ys", bufs=4) as ysp,
            tc.tile_pool(name="ps_s", bufs=2, space="PSUM") as ps_s,
            tc.tile_pool(name="ps_o", bufs=2, space="PSUM") as ps_o,
            tc.tile_pool(name="ps_m", bufs=2, space="PSUM") as ps_m,
        ):
            # ---------------- constants ----------------
            identf = constp.tile([128, 128], F32, tag="identf")
            make_identity(nc, identf[:])
            identb = constp.tile([128, 128], BF16, tag="identb")
            # additive transposed-causal bias: 0 where k <= q else -1e10
            # (accumulated into the scores PSUM via an identity matmul BEFORE
            # the score matmul, so exp sees s-1e10 -> 0 with no post-exp op)
            maskBf = constp.tile([128, 128], F32, tag="maskBf")
            nc.gpsimd.memset(maskBf[:], 0.0)
            nc.gpsimd.affine_select(
                out=maskBf[:],
                in_=maskBf[:],
                compare_op=mybir.AluOpType.is_ge,
                fill=-1e10,
                base=0,
                pattern=[[1, 128]],
                channel_multiplier=-1,
            )
            maskB = constp.tile([128, 128], BF16, tag="maskB")
            # e2: [64,128] selector; row0 -> out rows 0:64, row32 -> 64:128
            # (engine partition bases must be multiples of 32, so the two
            # denominator rows live at partitions 0 and 32)
            e2f = constp.tile([64, 128], F32, tag="e2f")
            nc.gpsimd.memset(e2f[:], 0.0)
            nc.gpsimd.memset(e2f[0:1, 0:64], 1.0)
            nc.gpsimd.memset(e2f[32:33, 64:128], 1.0)
            e2 = constp.tile([64, 128], F32R, tag="e2")
            onesf = constp.tile([128, NH], F32, tag="onesf")
            nc.gpsimd.memset(onesf[:], 1.0)
            onesb = constp.tile([128, NH], BF16, tag="onesb")
            zerof = constp.tile([128, 512], F32, tag="zerof")
            nc.gpsimd.memset(zerof[:], 0.0)
            with nc.allow_low_precision(reason="constant casts"):
                nc.vector.tensor_copy(maskB[:], maskBf[:])
                nc.vector.tensor_copy(identb[:], identf[:])
                nc.vector.tensor_copy(e2[:], e2f[:])
                nc.vector.tensor_copy(onesb[:], onesf[:])
            bq_t = constp.tile([128, NPAIR], F32, tag="bq")
            nc.gpsimd.dma_start(bq_t[:], bq_d.rearrange("c p -> p c"))

            # ---------------- weights / activations ----------------
            wqT = wp.tile([128, NFC, DL], F32R, tag="wqT")
            wqT_src = wqT_d.rearrange("(c p) d -> p c d", p=128).bitcast(F32R)
            for pc in range(NPAIR):
                nc.sync.dma_start(
                    wqT[:, :, pc * 128 : (pc + 1) * 128],
                    wqT_src[:, :, pc * 128 : (pc + 1) * 128],
                )
            # wpT is bf16 (host-converted): proj lhsT aT is bf16 and the
            # verifier requires matching dtypes when f32/f32r is involved.
            # On the gpsimd SWDGE ring: keeps both HWDGE queues free for the
            # startup-critical xT/wqT transfers (it's only needed at proj).
            wpT = wp.tile([128, NPAIR, NF], BF16, tag="wpT")
            nc.gpsimd.dma_start(
                wpT[:], wpT_d.rearrange("(c p) o -> p c o", p=128)
            )
            xT = wp.tile([128, NFC, NS], F32R, tag="xT")
            xT_src = xT_d.rearrange("(c p) t -> p c t", p=128).bitcast(F32R)
            # first transfers sized so the first qT matmuls start ASAP; only
            # the startup-critical pieces go on the Act (scalar) queue — later
            # DMA issues on Act would steal engine slots from the exp stream
            nc.scalar.dma_start(xT[:, 0:6, 0:256], xT_src[:, 0:6, 0:256])
            nc.scalar.dma_start(xT[:, :, 256:512], xT_src[:, :, 256:512])
            nc.sync.dma_start(xT[:, :, bass.ts(1, 512)], xT_src[:, :, bass.ts(1, 512)])
            nc.sync.dma_start(xT[:, :, bass.ts(2, 512)], xT_src[:, :, bass.ts(2, 512)])
            nc.sync.dma_start(xT[:, :, bass.ts(3, 512)], xT_src[:, :, bass.ts(3, 512)])

            # ---------------- interleaved: qT / V / attention per 512-chunk ----
            qT = bigp.tile([128, NPAIR, NS], BF16, tag="qT")
            # fp8 copy of qT for the off-diagonal score matmuls, laid out for
            # DoubleRow perf mode: [K=64, 2 k-tiles, cols] per head slice with
            # a ZERO second k-tile plane (out = q0^T k0 + 0^T 0, exact).
            # Numerically validated: off-diagonal scores in e4m3 with bf16
            # diagonal blocks gives 2.7e-3 rel err under faithful fp8
            # emulation (diagonal self-attention scores dominate the error
            # budget and stay bf16).
            qT8 = bigp.tile([128, NPAIR, 2, NS], F8, tag="qT8")
            for pc in range(NPAIR):
                nc.gpsimd.memset(qT8[:, pc, 1, :], 0.0)
            vt = bigp.tile([128, NKB, NH, HD + 1], BF16, tag="vt")
            aT = bigp.tile([128, NPAIR, NS], BF16, tag="aT")
            rs_tiles = []
            for pc in range(NPAIR):
                # [64, NS]: row 32*h2 holds 1/denom for head 2*pc+h2 (all q).
                # Zeroed ONCE here (DMA-wait time): rows other than 0/32 are
                # read by the e2 broadcast matmul multiplied by zero weights,
                # and garbage there could be NaN.
                # zeroed on the Act engine: it is idle during the startup DMA
                # window, while DVE sits on the first qT bias-add critical path
                rs_pc = bigp.tile([64, NS], F32R, tag=f"rs{pc}")
                for jc in range(NJC):
                    with nc.allow_low_precision(reason="f32r zeros"):
                        nc.scalar.copy(rs_pc[:, bass.ts(jc, 512)], zerof[0:64, :])
                rs_tiles.append(rs_pc)

            # k-block pair list per q-chunk jc: [(i0, i1), ...]
            def pairs_for(jc):
                nkb = 4 * jc + 4
                return [(i, i + 1) for i in range(0, nkb, 2)]

            # ---- deferred norm/proj work, woven between attention steps ----
            from collections import deque

            fill_q = deque()

            def pop_fill(n=1):
                for _ in range(min(n, len(fill_q))):
                    fill_q.popleft()()

            def make_norm_piece(jc, pc):
                def piece():
                    prb = ps_m.tile([128, 512], F32, tag="m", name="prb")
                    nc.tensor.matmul(
                        prb[:],
                        lhsT=e2[:],
                        rhs=rs_tiles[pc][:, bass.ts(jc, 512)],
                        start=True,
                        stop=True,
                    )
                    with nc.allow_low_precision(reason="bf16 normalize"):
                        nc.vector.tensor_tensor(
                            aT[:, pc, bass.ts(jc, 512)],
                            aT[:, pc, bass.ts(jc, 512)],
                            prb[:],
                            mybir.AluOpType.mult,
                        )

                return piece

            def make_proj_piece(tb, copies_on_act=False):
                def piece():
                    ysb = ysp.tile([128, NF], F32, tag="y", name="ysb")
                    for o0, on in ((0, 512), (512, 256)):
                        py = ps_m.tile([128, 512], F32, tag="m", name="py")
                        for pc in range(NPAIR):
                            nc.tensor.matmul(
                                py[:, 0:on],
                                lhsT=aT[:, pc, bass.ts(tb, 128)],
                                rhs=wpT[:, pc, o0 : o0 + on],
                                start=(pc == 0),
                                stop=(pc == NPAIR - 1),
                            )
                        if copies_on_act:
                            # the last chunk drains after the final exp: the
                            # Act engine is idle there while DVE is not
                            nc.scalar.copy(ysb[:, o0 : o0 + on], py[:, 0:on])
                        else:
                            nc.vector.tensor_copy(ysb[:, o0 : o0 + on], py[:, 0:on])
                    nc.sync.dma_start(y_d[bass.ts(tb, 128), :], ysb[:])

                return piece

            # qT production / V transposes as queueable pieces so chunk
            # jc+1's PE-heavy prep weaves into chunk jc's attention steps
            def make_pq_piece(tck, pc):
                def piece():
                    pq = ps_m.tile([128, 512], F32, tag="m", name="pq")
                    halves = ((0, 256), (256, 256)) if tck == 0 else ((0, 512),)
                    for c0, cn in halves:
                        for fc in range(NFC):
                            nc.tensor.matmul(
                                pq[:, c0 : c0 + cn],
                                lhsT=wqT[:, fc, pc * 128 : (pc + 1) * 128],
                                rhs=xT[:, fc, tck * 512 + c0 : tck * 512 + c0 + cn],
                                start=(fc == 0),
                                stop=(fc == NFC - 1),
                                skip_group_check=True,
                            )
                    with nc.allow_low_precision(reason="bf16 qT"):
                        nc.vector.tensor_scalar_add(
                            qT[:, pc, bass.ts(tck, 512)],
                            pq[:],
                            bq_t[:, pc : pc + 1],
                        )
                    with nc.allow_low_precision(reason="fp8 qT copy"):
                        nc.vector.tensor_scalar_add(
                            qT8[:, pc, 0, bass.ts(tck, 512)],
                            pq[:],
                            bq_t[:, pc : pc + 1],
                        )
                return piece

            def make_v_piece(i):
                # transpose via matmul with a bf16 identity as the MOVING
                # operand (out[k,d] = qT[d,k]; 128 cycles like a bf16
                # transpose, f32 PSUM in the shared "m" slots)
                def piece():
                    for pc in range(NPAIR):
                        pv = ps_m.tile([128, 512], F32, tag="m", name="pv")
                        nc.tensor.matmul(
                            pv[:, 0:128],
                            lhsT=qT[:, pc, bass.ts(i, 128)],
                            rhs=identb[:],
                            start=True,
                            stop=True,
                        )
                        with nc.allow_low_precision(reason="bf16 V"):
                            nc.vector.tensor_copy(
                                vt[:, i, 2 * pc : 2 * pc + 2, 0:HD],
                                pv[:, 0:128].rearrange("k (h d) -> k h d", h=2),
                            )
                    with nc.allow_low_precision(reason="ones col"):
                        nc.vector.tensor_copy(
                            vt[:, i, :, HD : HD + 1],
                            onesb[:].rearrange("p (h u) -> p h u", u=1),
                        )

                return piece

            def qv_pieces(tck):
                return [make_pq_piece(tck, pc) for pc in range(NPAIR)] + [
                    make_v_piece(i) for i in range(4 * tck, 4 * tck + 4)
                ]

            # prologue: chunk 0's qT + V emitted directly
            for p in qv_pieces(0):
                p()

            prep_q = deque()  # qT/V of the NEXT chunk: must drain by chunk end
            for tck in range(NJC):
                # ---- attention chunk jc = tck, paired k-blocks.  Emission is
                # software-pipelined for the in-order engine queues: each step
                # emits S+exp for pair g, then the AVs for pair g-1 (whose exp
                # and mask had a full step to complete), plus filler pieces
                # (next chunk's qT/V prep, prior chunks' norm/proj) to keep
                # PE fed and the exp stream continuous across chunks. ----
                jc = tck
                if tck + 1 < NJC:
                    prep_q.extend(qv_pieces(tck + 1))
                pend = {}  # h2 -> (pc, po, geom, pt2, last_of_pc)

                def flush_h2(h2):
                    if h2 not in pend:
                        return
                    f_pc, f_po, f_geom, f_pt, f_last = pend.pop(h2)
                    for i, off, c0, w in f_geom:
                        nc.tensor.matmul(
                            f_po[:, off:512],
                            lhsT=vt[:, i, 2 * f_pc + h2, :],
                            rhs=f_pt[:, c0 : c0 + w],
                            start=(i == 0),
                            stop=(i == 4 * jc + 3),
                            skip_group_check=True,
                        )
                    if f_last:
                        q_lo, q_hi = h2 * HD, (h2 + 1) * HD
                        with nc.allow_low_precision(reason="bf16 aT"):
                            nc.vector.tensor_copy(
                                aT[q_lo:q_hi, f_pc, bass.ts(jc, 512)],
                                f_po[0:HD, :],
                            )
                            nc.vector.reciprocal(
                                rs_tiles[f_pc][32 * h2 : 32 * h2 + 1, bass.ts(jc, 512)],
                                f_po[HD : HD + 1, :],
                            )
                        if h2 == 1:
                            # both heads of this pc reduced: normalize can
                            # weave into the remaining steps right away
                            fill_q.append(make_norm_piece(jc, f_pc))

                for pc in range(NPAIR):
                    po2 = [
                        ps_o.tile([HD + 1, 512], F32, tag="o", name=f"po{h2}")
                        for h2 in range(2)
                    ]
                    plist = pairs_for(jc)
                    for gi, (i0, i1) in enumerate(plist):
                        off0 = max(0, (i0 - 4 * jc) * 128)
                        off1 = max(0, (i1 - 4 * jc) * 128)
                        w0, w1 = 512 - off0, 512 - off1
                        geom = ((i0, off0, 0, w0), (i1, off1, w0, w1))
                        for h2 in (0, 1):
                            q_lo, q_hi = h2 * HD, (h2 + 1) * HD
                            ps2 = ps_s.tile([128, 1024], F32, tag="s")
                            pt2 = ptp.tile([128, 1024], BF16, tag="pt")
                            for i, off, c0, w in geom:
                                if i >= 4 * jc:
                                    # diagonal block: bias its first 128
                                    # stored cols with -1e10 above the
                                    # diagonal, then accumulate bf16 scores
                                    # on top (diagonal self-attention probs
                                    # dominate the error budget: keep bf16)
                                    nc.tensor.matmul(
                                        ps2[:, c0 : c0 + 128],
                                        lhsT=identb[:],
                                        rhs=maskB[:],
                                        start=True,
                                        stop=False,
                                        skip_group_check=True,
                                    )
                                    nc.tensor.matmul(
                                        ps2[:, c0 : c0 + 128],
                                        lhsT=qT[q_lo:q_hi, pc, bass.ts(i, 128)],
                                        rhs=qT[
                                            q_lo:q_hi,
                                            pc,
                                            jc * 512 + off : jc * 512 + off + 128,
                                        ],
                                        start=False,
                                        stop=True,
                                        skip_group_check=True,
                                    )
                                    if w > 128:
                                        nc.tensor.matmul(
                                            ps2[:, c0 + 128 : c0 + w],
                                            lhsT=qT8[q_lo:q_hi, pc, :, bass.ts(i, 128)],
                                            rhs=qT8[
                                                q_lo:q_hi,
                                                pc,
                                                :,
                                                jc * 512 + off + 128 : (jc + 1) * 512,
                                            ],
                                            start=True,
                                            stop=True,
                                            perf_mode=mybir.MatmulPerfMode.DoubleRow,
                                            skip_group_check=True,
                                        )
                                else:
                                    nc.tensor.matmul(
                                        ps2[:, c0 : c0 + w],
                                        lhsT=qT8[q_lo:q_hi, pc, :, bass.ts(i, 128)],
                                        rhs=qT8[
                                            q_lo:q_hi,
                                            pc,
                                            :,
                                            jc * 512 + off : (jc + 1) * 512,
                                        ],
                                        start=True,
                                        stop=True,
                                        perf_mode=mybir.MatmulPerfMode.DoubleRow,
                                        skip_group_check=True,
                                    )
                            with nc.allow_low_precision(reason="bf16 probs"):
                                nc.scalar.activation(
                                    pt2[:, 0 : w0 + w1],
                                    ps2[:, 0 : w0 + w1],
                                    mybir.ActivationFunctionType.Exp,
                                    scale=0.125,
                                )
                            flush_h2(h2)
                            pend[h2] = (pc, po2[h2], geom, pt2, gi == len(plist) - 1)
                        # weave filler: next chunk's prep first, then
                        # norm/proj of completed chunks
                        if prep_q:
                            prep_q.popleft()()
                        else:
                            pop_fill(1)
                flush_h2(0)
                flush_h2(1)
                # prep leftovers must finish before the next chunk's S reads qT
                while prep_q:
                    prep_q.popleft()()

                # proj of chunk jc needs all its norms queued/emittable
                for tb in range(4 * jc, 4 * jc + 4):
                    fill_q.append(make_proj_piece(tb, copies_on_act=(jc == NJC - 1)))

            # drain any remaining norm/proj work (chunk 3 + spillover)
            pop_fill(len(fill_q))

    nc.compile()
    return nc


def _build_passthrough():
    """I/O-identical no-compute kernel: isolates transfer+dispatch overhead
    so (wall(kernel) - wall(passthrough)) estimates device compute time."""
    import concourse.bacc as bacc
    import concourse.mybir as mybir

    F32 = mybir.dt.float32
    nc = bacc.Bacc(trn_type="TRN2", target_bir_lowering=False)
    xT_d = nc.dram_tensor("xT", [NF, NS], F32, kind="ExternalInput").ap()
    nc.dram_tensor("wqT", [NF, DL], F32, kind="ExternalInput").ap()
    nc.dram_tensor("bq", [NPAIR, 128], F32, kind="ExternalInput").ap()
    nc.dram_tensor("wpT", [DL, NF], mybir.dt.bfloat16, kind="ExternalInput").ap()
    y_d = nc.dram_tensor("y", [NS, NF], F32, kind="ExternalOutput").ap()
    # bounce same byte volume through SBUF
    import concourse.bass as bass
    import concourse.tile as tile

    xflat = xT_d.rearrange("a b -> (a b)").rearrange("(c x) -> c x", c=2048)
    with tile.TileContext(nc) as tc:
        with tc.tile_pool(name="sb", bufs=2) as sb:
            for i in range(16):
                t = sb.tile([128, 768], F32, tag="t")
                nc.sync.dma_start(t[:], xflat[bass.ts(i, 128), 0:768])
                nc.sync.dma_start(y_d[bass.ts(i, 128), :], t[:])
    nc.compile()
    return nc


def kernel(x, Wqkv_w, Wqkv_b, Wproj_w, Wproj_b, _passthrough=False):
    from concourse.bass_utils import run_bass_kernel_spmd

    x = np.asarray(x, dtype=np.float32)
    Wqkv_w = np.asarray(Wqkv_w, dtype=np.float32)
    Wqkv_b = np.asarray(Wqkv_b, dtype=np.float32)
    Wproj_w = np.asarray(Wproj_w, dtype=np.float32)
    Wproj_b = np.asarray(Wproj_b, dtype=np.float32)

    key = "nc_pt" if _passthrough else "nc"
    if key not in _COMPILED:
        _COMPILED[key] = _build_passthrough() if _passthrough else _build()
    nc = _COMPILED[key]

    import ml_dtypes

    in_maps = []
    for c in range(N_CORES):
        b, g = c // 2, c % 2
        sl = slice(g * DL, (g + 1) * DL)
        in_maps.append(
            {
                "xT": np.ascontiguousarray(x[b].T),
                "wqT": np.ascontiguousarray(Wqkv_w[:NF][sl].T),
                "bq": np.ascontiguousarray(Wqkv_b[:NF][sl].reshape(NPAIR, 128)),
                "wpT": np.ascontiguousarray(Wproj_w[:, sl].T).astype(
                    ml_dtypes.bfloat16
                ),
            }
        )

    trace = bool(int(os.environ.get("KERNEL_TRACE", "0")))
    res = run_bass_kernel_spmd(
        nc,
        in_maps,
        list(range(N_CORES)),
        trace=trace,
        trace_cores=list(range(N_CORES)) if trace else None,
    )
    if trace:
        _COMPILED["exec_time_ns"] = res.exec_time_ns
        _COMPILED["mean_exec_time_ns"] = res.mean_exec_time_ns
        _COMPILED["results_obj"] = res

    y = np.empty((NB, NS, NF), dtype=np.float32)
    for b in range(NB):
        y[b] = res.results[2 * b]["y"] + res.results[2 * b + 1]["y"] + Wproj_b
    return y


# revision 53
# speedup vs baseline: 1.0175x; 1.0175x over previous
"""TRN2 Bass kernel for nn_Attention_73839077752929.

Computes (matching the reference, which has the source bug k = v = q):
    q = x @ Wq^T + bq          (only the q-slice of Wqkv is ever used)
    a = softmax(causal(q q^T / 8)) @ q      per head
    y = a @ Wproj^T + bproj

Sharding: 8 cores = 4 batches x 2 head-groups (6 heads each).
Each core computes a partial projection output for its batch; the host
sums the two partials per batch and adds the projection bias.

On-core scheme (transposed-probability formulation, bf16 attention
operands, f32 PSUM accumulation):
    qT[d,t]   = wqT^T @ xT (+bias)  -> bf16   [head pairs on 128 partitions]
    V_i       = PE-transpose of qT (bf16), ones column pre-set
    PT        = exp((S^T)/8) per k-block PAIR (wide activations,
                multiplicative transposed-causal mask on diag blocks)
    OT'[d|1,q] = sum_i V_i^T @ PT_i    (extra row = softmax denominators)
    aT       *= bcast(1/denominators)   (deferred normalization)
    y[t,o]    = aT^T @ wpT
"""

import os

import numpy as np

N_CORES = 8
NB, NS, NF = 4, 2048, 768
N_HEADS_TOTAL = 12
HD = 64
NH = 6  # heads per core
DL = NH * HD  # 384 local dims
NPAIR = NH // 2  # 3 head pairs (128 partitions each)
NKB = NS // 128  # 16 k-blocks
NJC = NS // 512  # 4 q-chunks
NFC = NF // 128  # 6 feature chunks

_COMPILED = {}


def _build():
    import concourse.bacc as bacc
    import concourse.bass as bass
    import concourse.mybir as mybir
    import concourse.tile as tile
    from concourse.masks import make_identity

    F32 = mybir.dt.float32
    F32R = mybir.dt.float32r
    BF16 = mybir.dt.bfloat16
    F8 = mybir.dt.float8e4

    nc = bacc.Bacc(trn_type="TRN2", target_bir_lowering=False)

    xT_d = nc.dram_tensor("xT", [NF, NS], F32, kind="ExternalInput").ap()
    wqT_d = nc.dram_tensor("wqT", [NF, DL], F32, kind="ExternalInput").ap()
    bq_d = nc.dram_tensor("bq", [NPAIR, 128], F32, kind="ExternalInput").ap()
    wpT_d = nc.dram_tensor("wpT", [DL, NF], BF16, kind="ExternalInput").ap()
    y_d = nc.dram_tensor("y", [NS, NF], F32, kind="ExternalOutput").ap()

    with tile.TileContext(nc) as tc:
        with (
            tc.tile_pool(name="const", bufs=1) as constp,
            tc.tile_pool(name="w", bufs=1) as wp,
            tc.tile_pool(name="big", bufs=1) as bigp,
            tc.tile_pool(name="pt", bufs=7) as ptp,
            tc.tile_pool(name="ys", bufs=4) as ysp,
            tc.tile_pool(name="ps_s", bufs=2, space="PSUM") as ps_s,
            tc.tile_pool(name="ps_o", bufs=2, space="PSUM") as ps_o,
            tc.tile_pool(name="ps_m", bufs=2, space="PSUM") as ps_m,
        ):
            # ---------------- constants ----------------
            identf = constp.tile([128, 128], F32, tag="identf")
            make_identity(nc, identf[:])
            identb = constp.tile([128, 128], BF16, tag="identb")
            # additive transposed-causal bias: 0 where k <= q else -1e10
            # (accumulated into the scores PSUM via an identity matmul BEFORE
            # the score matmul, so exp sees s-1e10 -> 0 with no post-exp op)
            maskBf = constp.tile([128, 128], F32, tag="maskBf")
            nc.gpsimd.memset(maskBf[:], 0.0)
            nc.gpsimd.affine_select(
                out=maskBf[:],
                in_=maskBf[:],
                compare_op=mybir.AluOpType.is_ge,
                fill=-1e10,
                base=0,
                pattern=[[1, 128]],
                channel_multiplier=-1,
            )
            maskB = constp.tile([128, 128], BF16, tag="maskB")
            # e2: [64,128] selector; row0 -> out rows 0:64, row32 -> 64:128
            # (engine partition bases must be multiples of 32, so the two
            # denominator rows live at partitions 0 and 32)
            e2f = constp.tile([64, 128], F32, tag="e2f")
            nc.gpsimd.memset(e2f[:], 0.0)
            nc.gpsimd.memset(e2f[0:1, 0:64], 1.0)
            nc.gpsimd.memset(e2f[32:33, 64:128], 1.0)
            e2 = constp.tile([64, 128], F32R, tag="e2")
            onesf = constp.tile([128, NH], F32, tag="onesf")
            nc.gpsimd.memset(onesf[:], 1.0)
            onesb = constp.tile([128, NH], BF16, tag="onesb")
            zerof = constp.tile([128, 512], F32, tag="zerof")
            nc.gpsimd.memset(zerof[:], 0.0)
            with nc.allow_low_precision(reason="constant casts"):
                nc.vector.tensor_copy(maskB[:], maskBf[:])
                nc.vector.tensor_copy(identb[:], identf[:])
                nc.vector.tensor_copy(e2[:], e2f[:])
                nc.vector.tensor_copy(onesb[:], onesf[:])
            bq_t = constp.tile([128, NPAIR], F32, tag="bq")
            nc.gpsimd.dma_start(bq_t[:], bq_d.rearrange("c p -> p c"))

            # ---------------- weights / activations ----------------
            wqT = wp.tile([128, NFC, DL], F32R, tag="wqT")
            wqT_src = wqT_d.rearrange("(c p) d -> p c d", p=128).bitcast(F32R)
            for pc in range(NPAIR):
                nc.sync.dma_start(
                    wqT[:, :, pc * 128 : (pc + 1) * 128],
                    wqT_src[:, :, pc * 128 : (pc + 1) * 128],
                )
            # wpT is bf16 (host-converted): proj lhsT aT is bf16 and the
            # verifier requires matching dtypes when f32/f32r is involved.
            # On the gpsimd SWDGE ring: keeps both HWDGE queues free for the
            # startup-critical xT/wqT transfers (it's only needed at proj).
            wpT = wp.tile([128, NPAIR, NF], BF16, tag="wpT")
            nc.gpsimd.dma_start(
                wpT[:], wpT_d.rearrange("(c p) o -> p c o", p=128)
            )
            xT = wp.tile([128, NFC, NS], F32R, tag="xT")
            xT_src = xT_d.rearrange("(c p) t -> p c t", p=128).bitcast(F32R)
            # first transfers sized so the first qT matmuls start ASAP; only
            # the startup-critical pieces go on the Act (scalar) queue — later
            # DMA issues on Act would steal engine slots from the exp stream
            nc.scalar.dma_start(xT[:, 0:6, 0:256], xT_src[:, 0:6, 0:256])
            nc.scalar.dma_start(xT[:, :, 256:512], xT_src[:, :, 256:512])
            nc.sync.dma_start(xT[:, :, bass.ts(1, 512)], xT_src[:, :, bass.ts(1, 512)])
            nc.sync.dma_start(xT[:, :, bass.ts(2, 512)], xT_src[:, :, bass.ts(2, 512)])
            nc.sync.dma_start(xT[:, :, bass.ts(3, 512)], xT_src[:, :, bass.ts(3, 512)])

            # ---------------- interleaved: qT / V / attention per 512-chunk ----
            qT = bigp.tile([128, NPAIR, NS], BF16, tag="qT")
            # fp8 copy of qT for the off-diagonal score matmuls, laid out for
            # DoubleRow perf mode: [K=64, 2 k-tiles, cols] per head slice with
            # a ZERO second k-tile plane (out = q0^T k0 + 0^T 0, exact).
            # Numerically validated: off-diagonal scores in e4m3 with bf16
            # diagonal blocks gives 2.7e-3 rel err under faithful fp8
            # emulation (diagonal self-attention scores dominate the error
            # budget and stay bf16).
            qT8 = bigp.tile([128, NPAIR, 2, NS], F8, tag="qT8")
            for pc in range(NPAIR):
                nc.gpsimd.memset(qT8[:, pc, 1, :], 0.0)
            vt = bigp.tile([128, NKB, NH, HD + 1], BF16, tag="vt")
            aT = bigp.tile([128, NPAIR, NS], BF16, tag="aT")
            rs_tiles = []
            for pc in range(NPAIR):
                # [64, NS]: row 32*h2 holds 1/denom for head 2*pc+h2 (all q).
                # Zeroed ONCE here (DMA-wait time): rows other than 0/32 are
                # read by the e2 broadcast matmul multiplied by zero weights,
                # and garbage there could be NaN.
                # zeroed on the Act engine: it is idle during the startup DMA
                # window, while DVE sits on the first qT bias-add critical path
                rs_pc = bigp.tile([64, NS], F32R, tag=f"rs{pc}")
                for jc in range(NJC):
                    with nc.allow_low_precision(reason="f32r zeros"):
                        nc.scalar.copy(rs_pc[:, bass.ts(jc, 512)], zerof[0:64, :])
                rs_tiles.append(rs_pc)

            # k-block pair list per q-chunk jc: [(i0, i1), ...]
            def pairs_for(jc):
                nkb = 4 * jc + 4
                return [(i, i + 1) for i in range(0, nkb, 2)]

            # ---- deferred norm/proj work, woven between attention steps ----
            from collections import deque

            fill_q = deque()

            def pop_fill(n=1):
                for _ in range(min(n, len(fill_q))):
                    fill_q.popleft()()

            def make_norm_piece(jc, pc):
                def piece():
                    prb = ps_m.tile([128, 512], F32, tag="m", name="prb")
                    nc.tensor.matmul(
                        prb[:],
                        lhsT=e2[:],
                        rhs=rs_tiles[pc][:, bass.ts(jc, 512)],
                        start=True,
                        stop=True,
                    )
                    with nc.allow_low_precision(reason="bf16 normalize"):
                        nc.vector.tensor_tensor(
                            aT[:, pc, bass.ts(jc, 512)],
                            aT[:, pc, bass.ts(jc, 512)],
                            prb[:],
                            mybir.AluOpType.mult,
                        )

                return piece

            def make_proj_piece(tb, copies_on_act=False):
                def piece():
                    ysb = ysp.tile([128, NF], F32, tag="y", name="ysb")
                    for o0, on in ((0, 512), (512, 256)):
                        py = ps_m.tile([128, 512], F32, tag="m", name="py")
                        for pc in range(NPAIR):
                            nc.tensor.matmul(
                                py[:, 0:on],
                                lhsT=aT[:, pc, bass.ts(tb, 128)],
                                rhs=wpT[:, pc, o0 : o0 + on],
                                start=(pc == 0),
                                stop=(pc == NPAIR - 1),
                            )
                        if copies_on_act:
                            # the last chunk drains after the final exp: the
                            # Act engine is idle there while DVE is not
                            nc.scalar.copy(ysb[:, o0 : o0 + on], py[:, 0:on])
                        else:
                            nc.vector.tensor_copy(ysb[:, o0 : o0 + on], py[:, 0:on])
                    nc.sync.dma_start(y_d[bass.ts(tb, 128), :], ysb[:])

                return piece

            # qT production / V transposes as queueable pieces so chunk
            # jc+1's PE-heavy prep weaves into chunk jc's attention steps
            def make_pq_piece(tck, pc):
                def piece():
                    pq = ps_m.tile([128, 512], F32, tag="m", name="pq")
                    halves = ((0, 256), (256, 256)) if tck == 0 else ((0, 512),)
                    for c0, cn in halves:
                        for fc in range(NFC):
                            nc.tensor.matmul(
                                pq[:, c0 : c0 + cn],
                                lhsT=wqT[:, fc, pc * 128 : (pc + 1) * 128],
                                rhs=xT[:, fc, tck * 512 + c0 : tck * 512 + c0 + cn],
                                start=(fc == 0),
                                stop=(fc == NFC - 1),
                                skip_group_check=True,
                            )
                    with nc.allow_low_precision(reason="bf16 qT"):
                        nc.vector.tensor_scalar_add(
                            qT[:, pc, bass.ts(tck, 512)],
                            pq[:],
                            bq_t[:, pc : pc + 1],
                        )
                    with nc.allow_low_precision(reason="fp8 qT copy"):
                        nc.vector.tensor_scalar_add(
                            qT8[:, pc, 0, bass.ts(tck, 512)],
                            pq[:],
                            bq_t[:, pc : pc + 1],
                        )
                return piece

            def make_v_piece(i):
                # transpose via matmul with a bf16 identity as the MOVING
                # operand (out[k,d] = qT[d,k]; 128 cycles like a bf16
                # transpose, f32 PSUM in the shared "m" slots)
                def piece():
                    for pc in range(NPAIR):
                        pv = ps_m.tile([128, 512], F32, tag="m", name="pv")
                        nc.tensor.matmul(
                            pv[:, 0:128],
                            lhsT=qT[:, pc, bass.ts(i, 128)],
                            rhs=identb[:],
                            start=True,
                            stop=True,
                        )
                        with nc.allow_low_precision(reason="bf16 V"):
                            nc.vector.tensor_copy(
                                vt[:, i, 2 * pc : 2 * pc + 2, 0:HD],
                                pv[:, 0:128].rearrange("k (h d) -> k h d", h=2),
                            )
                    with nc.allow_low_precision(reason="ones col"):
                        nc.vector.tensor_copy(
                            vt[:, i, :, HD : HD + 1],
                            onesb[:].rearrange("p (h u) -> p h u", u=1),
                        )

                return piece

            def qv_pieces(tck):
                return [make_pq_piece(tck, pc) for pc in range(NPAIR)] + [
                    make_v_piece(i) for i in range(4 * tck, 4 * tck + 4)
                ]

            # prologue: chunk 0's qT + V emitted directly
            for p in qv_pieces(0):
                p()

            prep_q = deque()  # qT/V of the NEXT chunk: must drain by chunk end
            for tck in range(NJC):
                # ---- attention chunk jc = tck, paired k-blocks.  Emission is
                # software-pipelined for the in-order engine queues: each step
                # emits S+exp for pair g, then the AVs for pair g-1 (whose exp
                # and mask had a full step to complete), plus filler pieces
                # (next chunk's qT/V prep, prior chunks' norm/proj) to keep
                # PE fed and the exp stream continuous across chunks. ----
                jc = tck
                if tck + 1 < NJC:
                    prep_q.extend(qv_pieces(tck + 1))
                pend = {}  # h2 -> (pc, po, geom, pt2, last_of_pc)

                def flush_h2(h2):
                    if h2 not in pend:
                        return
                    f_pc, f_po, f_geom, f_pt, f_last = pend.pop(h2)
                    for i, off, c0, w in f_geom:
                        nc.tensor.matmul(
                            f_po[:, off:512],
                            lhsT=vt[:, i, 2 * f_pc + h2, :],
                            rhs=f_pt[:, c0 : c0 + w],
                            start=(i == 0),
                            stop=(i == 4 * jc + 3),
                            skip_group_check=True,
                        )
                    if f_last:
                        q_lo, q_hi = h2 * HD, (h2 + 1) * HD
                        with nc.allow_low_precision(reason="bf16 aT"):
                            nc.vector.tensor_copy(
                                aT[q_lo:q_hi, f_pc, bass.ts(jc, 512)],
                                f_po[0:HD, :],
                            )
                            nc.vector.reciprocal(
                                rs_tiles[f_pc][32 * h2 : 32 * h2 + 1, bass.ts(jc, 512)],
                                f_po[HD : HD + 1, :],
                            )
                        if h2 == 1:
                            # both heads of this pc reduced: normalize can
                            # weave into the remaining steps right away
                            fill_q.append(make_norm_piece(jc, f_pc))

                for pc in range(NPAIR):
                    po2 = [
                        ps_o.tile([HD + 1, 512], F32, tag="o", name=f"po{h2}")
                        for h2 in range(2)
                    ]
                    plist = pairs_for(jc)
                    for gi, (i0, i1) in enumerate(plist):
                        off0 = max(0, (i0 - 4 * jc) * 128)
                        off1 = max(0, (i1 - 4 * jc) * 128)
                        w0, w1 = 512 - off0, 512 - off1
                        geom = ((i0, off0, 0, w0), (i1, off1, w0, w1))
                        for h2 in (0, 1):
                            q_lo, q_hi = h2 * HD, (h2 + 1) * HD
                            ps2 = ps_s.tile([128, 1024], F32, tag="s")
                            pt2 = ptp.tile([128, 1024], BF16, tag="pt")
                            for i, off, c0, w in geom:
                                if i >= 4 * jc:
                                    # diagonal block: bias its first 128
                                    # stored cols with -1e10 above the
                                    # diagonal, then accumulate bf16 scores
                                    # on top (diagonal self-attention probs
                                    # dominate the error budget: keep bf16)
                                    nc.tensor.matmul(
                                        ps2[:, c0 : c0 + 128],
                                        lhsT=identb[:],
                                        rhs=maskB[:],
                                        start=True,
                                        stop=False,
                                        skip_group_check=True,
                                    )
                                    nc.tensor.matmul(
                                        ps2[:, c0 : c0 + 128],
                                        lhsT=qT[q_lo:q_hi, pc, bass.ts(i, 128)],
                                        rhs=qT[
                                            q_lo:q_hi,
                                            pc,
                                            jc * 512 + off : jc * 512 + off + 128,
                                        ],
                                        start=False,
                                        stop=True,
                                        skip_group_check=True,
                                    )
                                    if w > 128:
                                        nc.tensor.matmul(
                                            ps2[:, c0 + 128 : c0 + w],
                                            lhsT=qT8[q_lo:q_hi, pc, :, bass.ts(i, 128)],
                                            rhs=qT8[
                                                q_lo:q_hi,
                                                pc,
                                                :,
                                                jc * 512 + off + 128 : (jc + 1) * 512,
                                            ],
                                            start=True,
                                            stop=True,
                                            perf_mode=mybir.MatmulPerfMode.DoubleRow,
                                            skip_group_check=True,
                                        )
                                else:
                                    nc.tensor.matmul(
                                        ps2[:, c0 : c0 + w],
                                        lhsT=qT8[q_lo:q_hi, pc, :, bass.ts(i, 128)],
                                        rhs=qT8[
                                            q_lo:q_hi,
                                            pc,
                                            :,
                                            jc * 512 + off : (jc + 1) * 512,
                                        ],
                                        start=True,
                                        stop=True,
                                        perf_mode=mybir.MatmulPerfMode.DoubleRow,
                                        skip_group_check=True,
                                    )
                            with nc.allow_low_precision(reason="bf16 probs"):
                                nc.scalar.activation(
                                    pt2[:, 0 : w0 + w1],
                                    ps2[:, 0 : w0 + w1],
                                    mybir.ActivationFunctionType.Exp,
                                    scale=0.125,
                                )
                            flush_h2(h2)
                            pend[h2] = (pc, po2[h2], geom, pt2, gi == len(plist) - 1)
                        # weave filler: next chunk's prep first, then
                        # norm/proj of completed chunks
                        if prep_q:
                            prep_q.popleft()()
                        else:
                            pop_fill(1)
                flush_h2(0)
                flush_h2(1)
                # prep leftovers must finish before the next chunk's S reads qT
                while prep_q:
                    prep_q.popleft()()

                # proj of chunk jc needs all its norms queued/emittable
                for tb in range(4 * jc, 4 * jc + 4):
                    fill_q.append(make_proj_piece(tb))

            # drain any remaining norm/proj work (chunk 3 + spillover)
            pop_fill(len(fill_q))

    nc.compile()
    return nc


def _build_passthrough():
    """I/O-identical no-compute kernel: isolates transfer+dispatch overhead
    so (wall(kernel) - wall(passthrough)) estimates device compute time."""
    import concourse.bacc as bacc
    import concourse.mybir as mybir

    F32 = mybir.dt.float32
    nc = bacc.Bacc(trn_type="TRN2", target_bir_lowering=False)
    xT_d = nc.dram_tensor("xT", [NF, NS], F32, kind="ExternalInput").ap()
    nc.dram_tensor("wqT", [NF, DL], F32, kind="ExternalInput").ap()
    nc.dram_tensor("bq", [NPAIR, 128], F32, kind="ExternalInput").ap()
    nc.dram_tensor("wpT", [DL, NF], mybir.dt.bfloat16, kind="ExternalInput").ap()
    y_d = nc.dram_tensor("y", [NS, NF], F32, kind="ExternalOutput").ap()
    # bounce same byte volume through SBUF
    import concourse.bass as bass
    import concourse.tile as tile

    xflat = xT_d.rearrange("a b -> (a b)").rearrange("(c x) -> c x", c=2048)
    with tile.TileContext(nc) as tc:
        with tc.tile_pool(name="sb", bufs=2) as sb:
            for i in range(16):
                t = sb.tile([128, 768], F32, tag="t")
                nc.sync.dma_start(t[:], xflat[bass.ts(i, 128), 0:768])
                nc.sync.dma_start(y_d[bass.ts(i, 128), :], t[:])
    nc.compile()
    return nc


def kernel(x, Wqkv_w, Wqkv_b, Wproj_w, Wproj_b, _passthrough=False):
    from concourse.bass_utils import run_bass_kernel_spmd

    x = np.asarray(x, dtype=np.float32)
    Wqkv_w = np.asarray(Wqkv_w, dtype=np.float32)
    Wqkv_b = np.asarray(Wqkv_b, dtype=np.float32)
    Wproj_w = np.asarray(Wproj_w, dtype=np.float32)
    Wproj_b = np.asarray(Wproj_b, dtype=np.float32)

    key = "nc_pt" if _passthrough else "nc"
    if key not in _COMPILED:
        _COMPILED[key] = _build_passthrough() if _passthrough else _build()
    nc = _COMPILED[key]

    import ml_dtypes

    in_maps = []
    for c in range(N_CORES):
        b, g = c // 2, c % 2
        sl = slice(g * DL, (g + 1) * DL)
        in_maps.append(
            {
                "xT": np.ascontiguousarray(x[b].T),
                "wqT": np.ascontiguousarray(Wqkv_w[:NF][sl].T),
                "bq": np.ascontiguousarray(Wqkv_b[:NF][sl].reshape(NPAIR, 128)),
                "wpT": np.ascontiguousarray(Wproj_w[:, sl].T).astype(
                    ml_dtypes.bfloat16
                ),
            }
        )

    trace = bool(int(os.environ.get("KERNEL_TRACE", "0")))
    res = run_bass_kernel_spmd(
        nc,
        in_maps,
        list(range(N_CORES)),
        trace=trace,
        trace_cores=list(range(N_CORES)) if trace else None,
    )
    if trace:
        _COMPILED["exec_time_ns"] = res.exec_time_ns
        _COMPILED["mean_exec_time_ns"] = res.mean_exec_time_ns
        _COMPILED["results_obj"] = res

    y = np.empty((NB, NS, NF), dtype=np.float32)
    for b in range(NB):
        y[b] = res.results[2 * b]["y"] + res.results[2 * b + 1]["y"] + Wproj_b
    return y


# revision 55
# speedup vs baseline: 1.0210x; 1.0034x over previous
"""TRN2 Bass kernel for nn_Attention_73839077752929.

Computes (matching the reference, which has the source bug k = v = q):
    q = x @ Wq^T + bq          (only the q-slice of Wqkv is ever used)
    a = softmax(causal(q q^T / 8)) @ q      per head
    y = a @ Wproj^T + bproj

Sharding: 8 cores = 4 batches x 2 head-groups (6 heads each).
Each core computes a partial projection output for its batch; the host
sums the two partials per batch and adds the projection bias.

On-core scheme (transposed-probability formulation, bf16 attention
operands, f32 PSUM accumulation):
    qT[d,t]   = wqT^T @ xT (+bias)  -> bf16   [head pairs on 128 partitions]
    V_i       = PE-transpose of qT (bf16), ones column pre-set
    PT        = exp((S^T)/8) per k-block PAIR (wide activations,
                multiplicative transposed-causal mask on diag blocks)
    OT'[d|1,q] = sum_i V_i^T @ PT_i    (extra row = softmax denominators)
    aT       *= bcast(1/denominators)   (deferred normalization)
    y[t,o]    = aT^T @ wpT
"""

import os

import numpy as np

N_CORES = 8
NB, NS, NF = 4, 2048, 768
N_HEADS_TOTAL = 12
HD = 64
NH = 6  # heads per core
DL = NH * HD  # 384 local dims
NPAIR = NH // 2  # 3 head pairs (128 partitions each)
NKB = NS // 128  # 16 k-blocks
NJC = NS // 512  # 4 q-chunks
NFC = NF // 128  # 6 feature chunks

_COMPILED = {}


def _build():
    import concourse.bacc as bacc
    import concourse.bass as bass
    import concourse.mybir as mybir
    import concourse.tile as tile
    from concourse.masks import make_identity

    F32 = mybir.dt.float32
    F32R = mybir.dt.float32r
    BF16 = mybir.dt.bfloat16
    F8 = mybir.dt.float8e4

    nc = bacc.Bacc(trn_type="TRN2", target_bir_lowering=False)

    xT_d = nc.dram_tensor("xT", [NF, NS], F32, kind="ExternalInput").ap()
    wqT_d = nc.dram_tensor("wqT", [NF, DL], F32, kind="ExternalInput").ap()
    bq_d = nc.dram_tensor("bq", [NPAIR, 128], F32, kind="ExternalInput").ap()
    wpT_d = nc.dram_tensor("wpT", [DL, NF], BF16, kind="ExternalInput").ap()
    y_d = nc.dram_tensor("y", [NS, NF], F32, kind="ExternalOutput").ap()

    with tile.TileContext(nc) as tc:
        with (
            tc.tile_pool(name="const", bufs=1) as constp,
            tc.tile_pool(name="w", bufs=1) as wp,
            tc.tile_pool(name="big", bufs=1) as bigp,
            tc.tile_pool(name="pt", bufs=9) as ptp,
            tc.tile_pool(name="ys", bufs=4) as ysp,
            tc.tile_pool(name="ps_s", bufs=2, space="PSUM") as ps_s,
            tc.tile_pool(name="ps_o", bufs=2, space="PSUM") as ps_o,
            tc.tile_pool(name="ps_m", bufs=2, space="PSUM") as ps_m,
        ):
            # ---------------- constants ----------------
            identf = constp.tile([128, 128], F32, tag="identf")
            make_identity(nc, identf[:])
            identb = constp.tile([128, 128], BF16, tag="identb")
            # additive transposed-causal bias: 0 where k <= q else -1e10
            # (accumulated into the scores PSUM via an identity matmul BEFORE
            # the score matmul, so exp sees s-1e10 -> 0 with no post-exp op)
            maskBf = constp.tile([128, 128], F32, tag="maskBf")
            nc.gpsimd.memset(maskBf[:], 0.0)
            nc.gpsimd.affine_select(
                out=maskBf[:],
                in_=maskBf[:],
                compare_op=mybir.AluOpType.is_ge,
                fill=-1e10,
                base=0,
                pattern=[[1, 128]],
                channel_multiplier=-1,
            )
            maskB = constp.tile([128, 128], BF16, tag="maskB")
            # e2: [64,128] selector; row0 -> out rows 0:64, row32 -> 64:128
            # (engine partition bases must be multiples of 32, so the two
            # denominator rows live at partitions 0 and 32)
            e2f = constp.tile([64, 128], F32, tag="e2f")
            nc.gpsimd.memset(e2f[:], 0.0)
            nc.gpsimd.memset(e2f[0:1, 0:64], 1.0)
            nc.gpsimd.memset(e2f[32:33, 64:128], 1.0)
            e2 = constp.tile([64, 128], F32R, tag="e2")
            onesf = constp.tile([128, NH], F32, tag="onesf")
            nc.gpsimd.memset(onesf[:], 1.0)
            onesb = constp.tile([128, NH], BF16, tag="onesb")
            zerof = constp.tile([128, 512], F32, tag="zerof")
            nc.gpsimd.memset(zerof[:], 0.0)
            # casts on Act: it is idle during startup, DVE is on the first
            # qT bias-add critical path
            with nc.allow_low_precision(reason="constant casts"):
                nc.scalar.copy(maskB[:], maskBf[:])
                nc.scalar.copy(identb[:], identf[:])
                nc.scalar.copy(e2[:], e2f[:])
                nc.scalar.copy(onesb[:], onesf[:])
            bq_t = constp.tile([128, NPAIR], F32, tag="bq")
            nc.gpsimd.dma_start(bq_t[:], bq_d.rearrange("c p -> p c"))

            # ---------------- weights / activations ----------------
            wqT = wp.tile([128, NFC, DL], F32R, tag="wqT")
            wqT_src = wqT_d.rearrange("(c p) d -> p c d", p=128).bitcast(F32R)
            for pc in range(NPAIR):
                nc.sync.dma_start(
                    wqT[:, :, pc * 128 : (pc + 1) * 128],
                    wqT_src[:, :, pc * 128 : (pc + 1) * 128],
                )
            # wpT is bf16 (host-converted): proj lhsT aT is bf16 and the
            # verifier requires matching dtypes when f32/f32r is involved.
            # On the gpsimd SWDGE ring: keeps both HWDGE queues free for the
            # startup-critical xT/wqT transfers (it's only needed at proj).
            wpT = wp.tile([128, NPAIR, NF], BF16, tag="wpT")
            nc.gpsimd.dma_start(
                wpT[:], wpT_d.rearrange("(c p) o -> p c o", p=128)
            )
            xT = wp.tile([128, NFC, NS], F32R, tag="xT")
            xT_src = xT_d.rearrange("(c p) t -> p c t", p=128).bitcast(F32R)
            # first transfers sized so the first qT matmuls start ASAP; only
            # the startup-critical pieces go on the Act (scalar) queue — later
            # DMA issues on Act would steal engine slots from the exp stream
            nc.scalar.dma_start(xT[:, 0:6, 0:256], xT_src[:, 0:6, 0:256])
            nc.scalar.dma_start(xT[:, :, 256:512], xT_src[:, :, 256:512])
            nc.sync.dma_start(xT[:, :, bass.ts(1, 512)], xT_src[:, :, bass.ts(1, 512)])
            nc.sync.dma_start(xT[:, :, bass.ts(2, 512)], xT_src[:, :, bass.ts(2, 512)])
            nc.sync.dma_start(xT[:, :, bass.ts(3, 512)], xT_src[:, :, bass.ts(3, 512)])

            # ---------------- interleaved: qT / V / attention per 512-chunk ----
            qT = bigp.tile([128, NPAIR, NS], BF16, tag="qT")
            # fp8 copy of qT for the off-diagonal score matmuls, laid out for
            # DoubleRow perf mode: [K=64, 2 k-tiles, cols] per head slice with
            # a ZERO second k-tile plane (out = q0^T k0 + 0^T 0, exact).
            # Numerically validated: off-diagonal scores in e4m3 with bf16
            # diagonal blocks gives 2.7e-3 rel err under faithful fp8
            # emulation (diagonal self-attention scores dominate the error
            # budget and stay bf16).
            qT8 = bigp.tile([128, NPAIR, 2, NS], F8, tag="qT8")
            for pc in range(NPAIR):
                nc.gpsimd.memset(qT8[:, pc, 1, :], 0.0)
            vt = bigp.tile([128, NKB, NH, HD + 1], BF16, tag="vt")
            aT = bigp.tile([128, NPAIR, NS], BF16, tag="aT")
            rs_tiles = []
            for pc in range(NPAIR):
                # [64, NS]: row 32*h2 holds 1/denom for head 2*pc+h2 (all q).
                # Zeroed ONCE here (DMA-wait time): rows other than 0/32 are
                # read by the e2 broadcast matmul multiplied by zero weights,
                # and garbage there could be NaN.
                # zeroed on the Act engine: it is idle during the startup DMA
                # window, while DVE sits on the first qT bias-add critical path
                rs_pc = bigp.tile([64, NS], F32R, tag=f"rs{pc}")
                for jc in range(NJC):
                    with nc.allow_low_precision(reason="f32r zeros"):
                        nc.scalar.copy(rs_pc[:, bass.ts(jc, 512)], zerof[0:64, :])
                rs_tiles.append(rs_pc)

            # k-block pair list per q-chunk jc: [(i0, i1), ...]
            def pairs_for(jc):
                nkb = 4 * jc + 4
                return [(i, i + 1) for i in range(0, nkb, 2)]

            # ---- deferred norm/proj work, woven between attention steps ----
            from collections import deque

            fill_q = deque()

            def pop_fill(n=1):
                for _ in range(min(n, len(fill_q))):
                    fill_q.popleft()()

            def make_norm_piece(jc, pc):
                def piece():
                    prb = ps_m.tile([128, 512], F32, tag="m", name="prb")
                    nc.tensor.matmul(
                        prb[:],
                        lhsT=e2[:],
                        rhs=rs_tiles[pc][:, bass.ts(jc, 512)],
                        start=True,
                        stop=True,
                    )
                    with nc.allow_low_precision(reason="bf16 normalize"):
                        nc.vector.tensor_tensor(
                            aT[:, pc, bass.ts(jc, 512)],
                            aT[:, pc, bass.ts(jc, 512)],
                            prb[:],
                            mybir.AluOpType.mult,
                        )

                return piece

            def make_proj_piece(tb, copies_on_act=False):
                def piece():
                    ysb = ysp.tile([128, NF], F32, tag="y", name="ysb")
                    for o0, on in ((0, 512), (512, 256)):
                        py = ps_m.tile([128, 512], F32, tag="m", name="py")
                        for pc in range(NPAIR):
                            nc.tensor.matmul(
                                py[:, 0:on],
                                lhsT=aT[:, pc, bass.ts(tb, 128)],
                                rhs=wpT[:, pc, o0 : o0 + on],
                                start=(pc == 0),
                                stop=(pc == NPAIR - 1),
                            )
                        if copies_on_act:
                            # the last chunk drains after the final exp: the
                            # Act engine is idle there while DVE is not
                            nc.scalar.copy(ysb[:, o0 : o0 + on], py[:, 0:on])
                        else:
                            nc.vector.tensor_copy(ysb[:, o0 : o0 + on], py[:, 0:on])
                    nc.sync.dma_start(y_d[bass.ts(tb, 128), :], ysb[:])

                return piece

            # qT production / V transposes as queueable pieces so chunk
            # jc+1's PE-heavy prep weaves into chunk jc's attention steps
            def make_pq_piece(tck, pc):
                def piece():
                    pq = ps_m.tile([128, 512], F32, tag="m", name="pq")
                    halves = ((0, 256), (256, 256)) if tck == 0 else ((0, 512),)
                    for c0, cn in halves:
                        for fc in range(NFC):
                            nc.tensor.matmul(
                                pq[:, c0 : c0 + cn],
                                lhsT=wqT[:, fc, pc * 128 : (pc + 1) * 128],
                                rhs=xT[:, fc, tck * 512 + c0 : tck * 512 + c0 + cn],
                                start=(fc == 0),
                                stop=(fc == NFC - 1),
                                skip_group_check=True,
                            )
                    with nc.allow_low_precision(reason="bf16 qT"):
                        nc.vector.tensor_scalar_add(
                            qT[:, pc, bass.ts(tck, 512)],
                            pq[:],
                            bq_t[:, pc : pc + 1],
                        )
                    with nc.allow_low_precision(reason="fp8 qT copy"):
                        nc.vector.tensor_scalar_add(
                            qT8[:, pc, 0, bass.ts(tck, 512)],
                            pq[:],
                            bq_t[:, pc : pc + 1],
                        )
                return piece

            def make_v_piece(i):
                # transpose via matmul with a bf16 identity as the MOVING
                # operand (out[k,d] = qT[d,k]; 128 cycles like a bf16
                # transpose, f32 PSUM in the shared "m" slots)
                def piece():
                    for pc in range(NPAIR):
                        pv = ps_m.tile([128, 512], F32, tag="m", name="pv")
                        nc.tensor.matmul(
                            pv[:, 0:128],
                            lhsT=qT[:, pc, bass.ts(i, 128)],
                            rhs=identb[:],
                            start=True,
                            stop=True,
                        )
                        with nc.allow_low_precision(reason="bf16 V"):
                            nc.vector.tensor_copy(
                                vt[:, i, 2 * pc : 2 * pc + 2, 0:HD],
                                pv[:, 0:128].rearrange("k (h d) -> k h d", h=2),
                            )
                    with nc.allow_low_precision(reason="ones col"):
                        nc.vector.tensor_copy(
                            vt[:, i, :, HD : HD + 1],
                            onesb[:].rearrange("p (h u) -> p h u", u=1),
                        )

                return piece

            def qv_pieces(tck):
                return [make_pq_piece(tck, pc) for pc in range(NPAIR)] + [
                    make_v_piece(i) for i in range(4 * tck, 4 * tck + 4)
                ]

            # prologue: chunk 0's qT + V emitted directly
            for p in qv_pieces(0):
                p()

            prep_q = deque()  # qT/V of the NEXT chunk: must drain by chunk end
            for tck in range(NJC):
                # ---- attention chunk jc = tck, paired k-blocks.  Emission is
                # software-pipelined for the in-order engine queues: each step
                # emits S+exp for pair g, then the AVs for pair g-1 (whose exp
                # and mask had a full step to complete), plus filler pieces
                # (next chunk's qT/V prep, prior chunks' norm/proj) to keep
                # PE fed and the exp stream continuous across chunks. ----
                jc = tck
                if tck + 1 < NJC:
                    prep_q.extend(qv_pieces(tck + 1))
                pend = {}  # h2 -> (pc, po, geom, pt2, last_of_pc)

                def flush_h2(h2):
                    if h2 not in pend:
                        return
                    f_pc, f_po, f_geom, f_pt, f_last = pend.pop(h2)
                    for i, off, c0, w in f_geom:
                        nc.tensor.matmul(
                            f_po[:, off:512],
                            lhsT=vt[:, i, 2 * f_pc + h2, :],
                            rhs=f_pt[:, c0 : c0 + w],
                            start=(i == 0),
                            stop=(i == 4 * jc + 3),
                            skip_group_check=True,
                        )
                    if f_last:
                        q_lo, q_hi = h2 * HD, (h2 + 1) * HD
                        with nc.allow_low_precision(reason="bf16 aT"):
                            nc.vector.tensor_copy(
                                aT[q_lo:q_hi, f_pc, bass.ts(jc, 512)],
                                f_po[0:HD, :],
                            )
                            nc.vector.reciprocal(
                                rs_tiles[f_pc][32 * h2 : 32 * h2 + 1, bass.ts(jc, 512)],
                                f_po[HD : HD + 1, :],
                            )
                        if h2 == 1:
                            # both heads of this pc reduced: normalize can
                            # weave into the remaining steps right away
                            fill_q.append(make_norm_piece(jc, f_pc))

                for pc in range(NPAIR):
                    po2 = [
                        ps_o.tile([HD + 1, 512], F32, tag="o", name=f"po{h2}")
                        for h2 in range(2)
                    ]
                    plist = pairs_for(jc)
                    for gi, (i0, i1) in enumerate(plist):
                        off0 = max(0, (i0 - 4 * jc) * 128)
                        off1 = max(0, (i1 - 4 * jc) * 128)
                        w0, w1 = 512 - off0, 512 - off1
                        geom = ((i0, off0, 0, w0), (i1, off1, w0, w1))
                        for h2 in (0, 1):
                            q_lo, q_hi = h2 * HD, (h2 + 1) * HD
                            ps2 = ps_s.tile([128, 1024], F32, tag="s")
                            pt2 = ptp.tile([128, 1024], BF16, tag="pt")
                            for i, off, c0, w in geom:
                                if i >= 4 * jc:
                                    # diagonal block: bias its first 128
                                    # stored cols with -1e10 above the
                                    # diagonal, then accumulate bf16 scores
                                    # on top (diagonal self-attention probs
                                    # dominate the error budget: keep bf16)
                                    nc.tensor.matmul(
                                        ps2[:, c0 : c0 + 128],
                                        lhsT=identb[:],
                                        rhs=maskB[:],
                                        start=True,
                                        stop=False,
                                        skip_group_check=True,
                                    )
                                    nc.tensor.matmul(
                                        ps2[:, c0 : c0 + 128],
                                        lhsT=qT[q_lo:q_hi, pc, bass.ts(i, 128)],
                                        rhs=qT[
                                            q_lo:q_hi,
                                            pc,
                                            jc * 512 + off : jc * 512 + off + 128,
                                        ],
                                        start=False,
                                        stop=True,
                                        skip_group_check=True,
                                    )
                                    if w > 128:
                                        nc.tensor.matmul(
                                            ps2[:, c0 + 128 : c0 + w],
                                            lhsT=qT8[q_lo:q_hi, pc, :, bass.ts(i, 128)],
                                            rhs=qT8[
                                                q_lo:q_hi,
                                                pc,
                                                :,
                                                jc * 512 + off + 128 : (jc + 1) * 512,
                                            ],
                                            start=True,
                                            stop=True,
                                            perf_mode=mybir.MatmulPerfMode.DoubleRow,
                                            skip_group_check=True,
                                        )
                                else:
                                    nc.tensor.matmul(
                                        ps2[:, c0 : c0 + w],
                                        lhsT=qT8[q_lo:q_hi, pc, :, bass.ts(i, 128)],
                                        rhs=qT8[
                                            q_lo:q_hi,
                                            pc,
                                            :,
                                            jc * 512 + off : (jc + 1) * 512,
                                        ],
                                        start=True,
                                        stop=True,
                                        perf_mode=mybir.MatmulPerfMode.DoubleRow,
                                        skip_group_check=True,
                                    )
                            with nc.allow_low_precision(reason="bf16 probs"):
                                nc.scalar.activation(
                                    pt2[:, 0 : w0 + w1],
                                    ps2[:, 0 : w0 + w1],
                                    mybir.ActivationFunctionType.Exp,
                                    scale=0.125,
                                )
                            flush_h2(h2)
                            pend[h2] = (pc, po2[h2], geom, pt2, gi == len(plist) - 1)
                        # weave filler: next chunk's prep first, then
                        # norm/proj of completed chunks
                        if prep_q:
                            prep_q.popleft()()
                        else:
                            pop_fill(1)
                flush_h2(0)
                flush_h2(1)
                # prep leftovers must finish before the next chunk's S reads qT
                while prep_q:
                    prep_q.popleft()()

                # proj of chunk jc needs all its norms queued/emittable
                for tb in range(4 * jc, 4 * jc + 4):
                    fill_q.append(make_proj_piece(tb))

            # drain any remaining norm/proj work (chunk 3 + spillover)
            pop_fill(len(fill_q))

    nc.compile()
    return nc


def _build_passthrough():
    """I/O-identical no-compute kernel: isolates transfer+dispatch overhead
    so (wall(kernel) - wall(passthrough)) estimates device compute time."""
    import concourse.bacc as bacc
    import concourse.mybir as mybir

    F32 = mybir.dt.float32
    nc = bacc.Bacc(trn_type="TRN2", target_bir_lowering=False)
    xT_d = nc.dram_tensor("xT", [NF, NS], F32, kind="ExternalInput").ap()
    nc.dram_tensor("wqT", [NF, DL], F32, kind="ExternalInput").ap()
    nc.dram_tensor("bq", [NPAIR, 128], F32, kind="ExternalInput").ap()
    nc.dram_tensor("wpT", [DL, NF], mybir.dt.bfloat16, kind="ExternalInput").ap()
    y_d = nc.dram_tensor("y", [NS, NF], F32, kind="ExternalOutput").ap()
    # bounce same byte volume through SBUF
    import concourse.bass as bass
    import concourse.tile as tile

    xflat = xT_d.rearrange("a b -> (a b)").rearrange("(c x) -> c x", c=2048)
    with tile.TileContext(nc) as tc:
        with tc.tile_pool(name="sb", bufs=2) as sb:
            for i in range(16):
                t = sb.tile([128, 768], F32, tag="t")
                nc.sync.dma_start(t[:], xflat[bass.ts(i, 128), 0:768])
                nc.sync.dma_start(y_d[bass.ts(i, 128), :], t[:])
    nc.compile()
    return nc


def kernel(x, Wqkv_w, Wqkv_b, Wproj_w, Wproj_b, _passthrough=False):
    from concourse.bass_utils import run_bass_kernel_spmd

    x = np.asarray(x, dtype=np.float32)
    Wqkv_w = np.asarray(Wqkv_w, dtype=np.float32)
    Wqkv_b = np.asarray(Wqkv_b, dtype=np.float32)
    Wproj_w = np.asarray(Wproj_w, dtype=np.float32)
    Wproj_b = np.asarray(Wproj_b, dtype=np.float32)

    key = "nc_pt" if _passthrough else "nc"
    if key not in _COMPILED:
        _COMPILED[key] = _build_passthrough() if _passthrough else _build()
    nc = _COMPILED[key]

    import ml_dtypes

    in_maps = []
    for c in range(N_CORES):
        b, g = c // 2, c % 2
        sl = slice(g * DL, (g + 1) * DL)
        in_maps.append(
            {
                "xT": np.ascontiguousarray(x[b].T),
                "wqT": np.ascontiguousarray(Wqkv_w[:NF][sl].T),
                "bq": np.ascontiguousarray(Wqkv_b[:NF][sl].reshape(NPAIR, 128)),
                "wpT": np.ascontiguousarray(Wproj_w[:, sl].T).astype(
                    ml_dtypes.bfloat16
                ),
            }
        )

    trace = bool(int(os.environ.get("KERNEL_TRACE", "0")))
    res = run_bass_kernel_spmd(
        nc,
        in_maps,
        list(range(N_CORES)),
        trace=trace,
        trace_cores=list(range(N_CORES)) if trace else None,
    )
    if trace:
        _COMPILED["exec_time_ns"] = res.exec_time_ns
        _COMPILED["mean_exec_time_ns"] = res.mean_exec_time_ns
        _COMPILED["results_obj"] = res

    y = np.empty((NB, NS, NF), dtype=np.float32)
    for b in range(NB):
        y[b] = res.results[2 * b]["y"] + res.results[2 * b + 1]["y"] + Wproj_b
    return y


# revision 58
# speedup vs baseline: 1.0311x; 1.0099x over previous
"""TRN2 Bass kernel for nn_Attention_73839077752929.

Computes (matching the reference, which has the source bug k = v = q):
    q = x @ Wq^T + bq          (only the q-slice of Wqkv is ever used)
    a = softmax(causal(q q^T / 8)) @ q      per head
    y = a @ Wproj^T + bproj

Sharding: 8 cores = 4 batches x 2 head-groups (6 heads each).
Each core computes a partial projection output for its batch; the host
sums the two partials per batch and adds the projection bias.

On-core scheme (transposed-probability formulation, bf16 attention
operands, f32 PSUM accumulation):
    qT[d,t]   = wqT^T @ xT (+bias)  -> bf16   [head pairs on 128 partitions]
    V_i       = PE-transpose of qT (bf16), ones column pre-set
    PT        = exp((S^T)/8) per k-block PAIR (wide activations,
                multiplicative transposed-causal mask on diag blocks)
    OT'[d|1,q] = sum_i V_i^T @ PT_i    (extra row = softmax denominators)
    aT       *= bcast(1/denominators)   (deferred normalization)
    y[t,o]    = aT^T @ wpT
"""

import os

import numpy as np

N_CORES = 8
NB, NS, NF = 4, 2048, 768
N_HEADS_TOTAL = 12
HD = 64
NH = 6  # heads per core
DL = NH * HD  # 384 local dims
NPAIR = NH // 2  # 3 head pairs (128 partitions each)
NKB = NS // 128  # 16 k-blocks
NJC = NS // 512  # 4 q-chunks
NFC = NF // 128  # 6 feature chunks

_COMPILED = {}


def _build():
    import concourse.bacc as bacc
    import concourse.bass as bass
    import concourse.mybir as mybir
    import concourse.tile as tile
    from concourse.masks import make_identity

    F32 = mybir.dt.float32
    F32R = mybir.dt.float32r
    BF16 = mybir.dt.bfloat16
    F8 = mybir.dt.float8e4

    nc = bacc.Bacc(trn_type="TRN2", target_bir_lowering=False)

    xT_d = nc.dram_tensor("xT", [NF, NS], F32, kind="ExternalInput").ap()
    wqT_d = nc.dram_tensor("wqT", [NF, DL], F32, kind="ExternalInput").ap()
    bq_d = nc.dram_tensor("bq", [NPAIR, 128], F32, kind="ExternalInput").ap()
    wpT_d = nc.dram_tensor("wpT", [DL, NF], BF16, kind="ExternalInput").ap()
    y_d = nc.dram_tensor("y", [NS, NF], F32, kind="ExternalOutput").ap()

    with tile.TileContext(nc) as tc:
        with (
            tc.tile_pool(name="const", bufs=1) as constp,
            tc.tile_pool(name="w", bufs=1) as wp,
            tc.tile_pool(name="big", bufs=1) as bigp,
            tc.tile_pool(name="pt", bufs=9) as ptp,
            tc.tile_pool(name="ys", bufs=4) as ysp,
            tc.tile_pool(name="ps_s", bufs=2, space="PSUM") as ps_s,
            tc.tile_pool(name="ps_o", bufs=2, space="PSUM") as ps_o,
            tc.tile_pool(name="ps_m", bufs=2, space="PSUM") as ps_m,
        ):
            # ---------------- constants ----------------
            identf = constp.tile([128, 128], F32, tag="identf")
            make_identity(nc, identf[:])
            identb = constp.tile([128, 128], BF16, tag="identb")
            # additive transposed-causal bias: 0 where k <= q else -1e10
            # (accumulated into the scores PSUM via an identity matmul BEFORE
            # the score matmul, so exp sees s-1e10 -> 0 with no post-exp op)
            maskBf = constp.tile([128, 128], F32, tag="maskBf")
            nc.gpsimd.memset(maskBf[:], 0.0)
            nc.gpsimd.affine_select(
                out=maskBf[:],
                in_=maskBf[:],
                compare_op=mybir.AluOpType.is_ge,
                fill=-1e10,
                base=0,
                pattern=[[1, 128]],
                channel_multiplier=-1,
            )
            maskB = constp.tile([128, 128], BF16, tag="maskB")
            # e2: [64,128] selector; row0 -> out rows 0:64, row32 -> 64:128
            # (engine partition bases must be multiples of 32, so the two
            # denominator rows live at partitions 0 and 32)
            e2f = constp.tile([64, 128], F32, tag="e2f")
            nc.gpsimd.memset(e2f[:], 0.0)
            nc.gpsimd.memset(e2f[0:1, 0:64], 1.0)
            nc.gpsimd.memset(e2f[32:33, 64:128], 1.0)
            e2 = constp.tile([64, 128], F32R, tag="e2")
            onesf = constp.tile([128, NH], F32, tag="onesf")
            nc.gpsimd.memset(onesf[:], 1.0)
            onesb = constp.tile([128, NH], BF16, tag="onesb")
            zerof = constp.tile([128, 512], F32, tag="zerof")
            nc.gpsimd.memset(zerof[:], 0.0)
            # casts on Act: it is idle during startup, DVE is on the first
            # qT bias-add critical path
            with nc.allow_low_precision(reason="constant casts"):
                nc.scalar.copy(maskB[:], maskBf[:])
                nc.scalar.copy(identb[:], identf[:])
                nc.scalar.copy(e2[:], e2f[:])
                nc.scalar.copy(onesb[:], onesf[:])
            bq_t = constp.tile([128, NPAIR], F32, tag="bq")
            nc.gpsimd.dma_start(bq_t[:], bq_d.rearrange("c p -> p c"))

            # ---------------- weights / activations ----------------
            wqT = wp.tile([128, NFC, DL], F32R, tag="wqT")
            wqT_src = wqT_d.rearrange("(c p) d -> p c d", p=128).bitcast(F32R)
            for pc in range(NPAIR):
                nc.sync.dma_start(
                    wqT[:, :, pc * 128 : (pc + 1) * 128],
                    wqT_src[:, :, pc * 128 : (pc + 1) * 128],
                )
            # wpT is bf16 (host-converted): proj lhsT aT is bf16 and the
            # verifier requires matching dtypes when f32/f32r is involved.
            # On the gpsimd SWDGE ring: keeps both HWDGE queues free for the
            # startup-critical xT/wqT transfers (it's only needed at proj).
            wpT = wp.tile([128, NPAIR, NF], BF16, tag="wpT")
            nc.gpsimd.dma_start(
                wpT[:], wpT_d.rearrange("(c p) o -> p c o", p=128)
            )
            xT = wp.tile([128, NFC, NS], F32R, tag="xT")
            xT_src = xT_d.rearrange("(c p) t -> p c t", p=128).bitcast(F32R)
            # first transfers sized so the first qT matmuls start ASAP; only
            # the startup-critical pieces go on the Act (scalar) queue — later
            # DMA issues on Act would steal engine slots from the exp stream
            nc.scalar.dma_start(xT[:, 0:6, 0:256], xT_src[:, 0:6, 0:256])
            nc.scalar.dma_start(xT[:, :, 256:512], xT_src[:, :, 256:512])
            nc.sync.dma_start(xT[:, :, bass.ts(1, 512)], xT_src[:, :, bass.ts(1, 512)])
            nc.sync.dma_start(xT[:, :, bass.ts(2, 512)], xT_src[:, :, bass.ts(2, 512)])
            nc.sync.dma_start(xT[:, :, bass.ts(3, 512)], xT_src[:, :, bass.ts(3, 512)])

            # ---------------- interleaved: qT / V / attention per 512-chunk ----
            qT = bigp.tile([128, NPAIR, NS], BF16, tag="qT")
            # fp8 copy of qT for the off-diagonal score matmuls, laid out for
            # DoubleRow perf mode: [K=64, 2 k-tiles, cols] per head slice with
            # a ZERO second k-tile plane (out = q0^T k0 + 0^T 0, exact).
            # Numerically validated: off-diagonal scores in e4m3 with bf16
            # diagonal blocks gives 2.7e-3 rel err under faithful fp8
            # emulation (diagonal self-attention scores dominate the error
            # budget and stay bf16).
            qT8 = bigp.tile([128, NPAIR, 2, NS], F8, tag="qT8")
            for pc in range(NPAIR):
                nc.gpsimd.memset(qT8[:, pc, 1, :], 0.0)
            vt = bigp.tile([128, NKB, NH, HD + 1], BF16, tag="vt")
            aT = bigp.tile([128, NPAIR, NS], BF16, tag="aT")
            rs_tiles = []
            for pc in range(NPAIR):
                # [64, NS]: row 32*h2 holds 1/denom for head 2*pc+h2 (all q).
                # Zeroed ONCE here (DMA-wait time): rows other than 0/32 are
                # read by the e2 broadcast matmul multiplied by zero weights,
                # and garbage there could be NaN.
                # zeroed on the Act engine: it is idle during the startup DMA
                # window, while DVE sits on the first qT bias-add critical path
                rs_pc = bigp.tile([64, NS], F32R, tag=f"rs{pc}")
                for jc in range(NJC):
                    with nc.allow_low_precision(reason="f32r zeros"):
                        nc.scalar.copy(rs_pc[:, bass.ts(jc, 512)], zerof[0:64, :])
                rs_tiles.append(rs_pc)

            # k-block pair list per q-chunk jc: [(i0, i1), ...]
            def pairs_for(jc):
                nkb = 4 * jc + 4
                return [(i, i + 1) for i in range(0, nkb, 2)]

            # ---- deferred norm/proj work, woven between attention steps ----
            from collections import deque

            fill_q = deque()

            def pop_fill(n=1):
                for _ in range(min(n, len(fill_q))):
                    fill_q.popleft()()

            def make_norm_piece(jc, pc):
                def piece():
                    prb = ps_m.tile([128, 512], F32, tag="m", name="prb")
                    nc.tensor.matmul(
                        prb[:],
                        lhsT=e2[:],
                        rhs=rs_tiles[pc][:, bass.ts(jc, 512)],
                        start=True,
                        stop=True,
                    )
                    with nc.allow_low_precision(reason="bf16 normalize"):
                        nc.vector.tensor_tensor(
                            aT[:, pc, bass.ts(jc, 512)],
                            aT[:, pc, bass.ts(jc, 512)],
                            prb[:],
                            mybir.AluOpType.mult,
                        )

                return piece

            def make_proj_piece(tb, copies_on_act=False):
                def piece():
                    ysb = ysp.tile([128, NF], F32, tag="y", name="ysb")
                    for o0, on in ((0, 512), (512, 256)):
                        py = ps_m.tile([128, 512], F32, tag="m", name="py")
                        for pc in range(NPAIR):
                            nc.tensor.matmul(
                                py[:, 0:on],
                                lhsT=aT[:, pc, bass.ts(tb, 128)],
                                rhs=wpT[:, pc, o0 : o0 + on],
                                start=(pc == 0),
                                stop=(pc == NPAIR - 1),
                            )
                        if copies_on_act and o0 == 512:
                            # last-chunk pieces drain after the final exp:
                            # Act is idle there, so the small half copies in
                            # parallel with DVE doing the 512-wide half
                            nc.scalar.copy(ysb[:, o0 : o0 + on], py[:, 0:on])
                        else:
                            nc.vector.tensor_copy(ysb[:, o0 : o0 + on], py[:, 0:on])
                    nc.sync.dma_start(y_d[bass.ts(tb, 128), :], ysb[:])

                return piece

            # qT production / V transposes as queueable pieces so chunk
            # jc+1's PE-heavy prep weaves into chunk jc's attention steps
            def make_pq_piece(tck, pc):
                def piece():
                    pq = ps_m.tile([128, 512], F32, tag="m", name="pq")
                    halves = ((0, 256), (256, 256)) if tck == 0 else ((0, 512),)
                    for c0, cn in halves:
                        for fc in range(NFC):
                            nc.tensor.matmul(
                                pq[:, c0 : c0 + cn],
                                lhsT=wqT[:, fc, pc * 128 : (pc + 1) * 128],
                                rhs=xT[:, fc, tck * 512 + c0 : tck * 512 + c0 + cn],
                                start=(fc == 0),
                                stop=(fc == NFC - 1),
                                skip_group_check=True,
                            )
                    with nc.allow_low_precision(reason="bf16 qT"):
                        nc.vector.tensor_scalar_add(
                            qT[:, pc, bass.ts(tck, 512)],
                            pq[:],
                            bq_t[:, pc : pc + 1],
                        )
                    with nc.allow_low_precision(reason="fp8 qT copy"):
                        nc.vector.tensor_scalar_add(
                            qT8[:, pc, 0, bass.ts(tck, 512)],
                            pq[:],
                            bq_t[:, pc : pc + 1],
                        )
                return piece

            def make_v_piece(i):
                # transpose via matmul with a bf16 identity as the MOVING
                # operand (out[k,d] = qT[d,k]; 128 cycles like a bf16
                # transpose, f32 PSUM in the shared "m" slots)
                def piece():
                    for pc in range(NPAIR):
                        pv = ps_m.tile([128, 512], F32, tag="m", name="pv")
                        nc.tensor.matmul(
                            pv[:, 0:128],
                            lhsT=qT[:, pc, bass.ts(i, 128)],
                            rhs=identb[:],
                            start=True,
                            stop=True,
                        )
                        with nc.allow_low_precision(reason="bf16 V"):
                            nc.vector.tensor_copy(
                                vt[:, i, 2 * pc : 2 * pc + 2, 0:HD],
                                pv[:, 0:128].rearrange("k (h d) -> k h d", h=2),
                            )
                    with nc.allow_low_precision(reason="ones col"):
                        nc.vector.tensor_copy(
                            vt[:, i, :, HD : HD + 1],
                            onesb[:].rearrange("p (h u) -> p h u", u=1),
                        )

                return piece

            def qv_pieces(tck):
                return [make_pq_piece(tck, pc) for pc in range(NPAIR)] + [
                    make_v_piece(i) for i in range(4 * tck, 4 * tck + 4)
                ]

            # prologue: chunk 0's qT + V emitted directly
            for p in qv_pieces(0):
                p()

            prep_q = deque()  # qT/V of the NEXT chunk: must drain by chunk end
            for tck in range(NJC):
                # ---- attention chunk jc = tck, paired k-blocks.  Emission is
                # software-pipelined for the in-order engine queues: each step
                # emits S+exp for pair g, then the AVs for pair g-1 (whose exp
                # and mask had a full step to complete), plus filler pieces
                # (next chunk's qT/V prep, prior chunks' norm/proj) to keep
                # PE fed and the exp stream continuous across chunks. ----
                jc = tck
                if tck + 1 < NJC:
                    prep_q.extend(qv_pieces(tck + 1))
                pend = {}  # h2 -> (pc, po, geom, pt2, last_of_pc)

                def flush_h2(h2):
                    if h2 not in pend:
                        return
                    f_pc, f_po, f_geom, f_pt, f_last = pend.pop(h2)
                    for i, off, c0, w in f_geom:
                        nc.tensor.matmul(
                            f_po[:, off:512],
                            lhsT=vt[:, i, 2 * f_pc + h2, :],
                            rhs=f_pt[:, c0 : c0 + w],
                            start=(i == 0),
                            stop=(i == 4 * jc + 3),
                            skip_group_check=True,
                        )
                    if f_last:
                        q_lo, q_hi = h2 * HD, (h2 + 1) * HD
                        with nc.allow_low_precision(reason="bf16 aT"):
                            nc.vector.tensor_copy(
                                aT[q_lo:q_hi, f_pc, bass.ts(jc, 512)],
                                f_po[0:HD, :],
                            )
                            nc.vector.reciprocal(
                                rs_tiles[f_pc][32 * h2 : 32 * h2 + 1, bass.ts(jc, 512)],
                                f_po[HD : HD + 1, :],
                            )
                        if h2 == 1:
                            # both heads of this pc reduced: normalize can
                            # weave into the remaining steps right away
                            fill_q.append(make_norm_piece(jc, f_pc))

                for pc in range(NPAIR):
                    po2 = [
                        ps_o.tile([HD + 1, 512], F32, tag="o", name=f"po{h2}")
                        for h2 in range(2)
                    ]
                    plist = pairs_for(jc)
                    for gi, (i0, i1) in enumerate(plist):
                        off0 = max(0, (i0 - 4 * jc) * 128)
                        off1 = max(0, (i1 - 4 * jc) * 128)
                        w0, w1 = 512 - off0, 512 - off1
                        geom = ((i0, off0, 0, w0), (i1, off1, w0, w1))
                        for h2 in (0, 1):
                            q_lo, q_hi = h2 * HD, (h2 + 1) * HD
                            ps2 = ps_s.tile([128, 1024], F32, tag="s")
                            pt2 = ptp.tile([128, 1024], BF16, tag="pt")
                            for i, off, c0, w in geom:
                                if i >= 4 * jc:
                                    # diagonal block: bias its first 128
                                    # stored cols with -1e10 above the
                                    # diagonal, then accumulate bf16 scores
                                    # on top (diagonal self-attention probs
                                    # dominate the error budget: keep bf16)
                                    nc.tensor.matmul(
                                        ps2[:, c0 : c0 + 128],
                                        lhsT=identb[:],
                                        rhs=maskB[:],
                                        start=True,
                                        stop=False,
                                        skip_group_check=True,
                                    )
                                    nc.tensor.matmul(
                                        ps2[:, c0 : c0 + 128],
                                        lhsT=qT[q_lo:q_hi, pc, bass.ts(i, 128)],
                                        rhs=qT[
                                            q_lo:q_hi,
                                            pc,
                                            jc * 512 + off : jc * 512 + off + 128,
                                        ],
                                        start=False,
                                        stop=True,
                                        skip_group_check=True,
                                    )
                                    if w > 128:
                                        nc.tensor.matmul(
                                            ps2[:, c0 + 128 : c0 + w],
                                            lhsT=qT8[q_lo:q_hi, pc, :, bass.ts(i, 128)],
                                            rhs=qT8[
                                                q_lo:q_hi,
                                                pc,
                                                :,
                                                jc * 512 + off + 128 : (jc + 1) * 512,
                                            ],
                                            start=True,
                                            stop=True,
                                            perf_mode=mybir.MatmulPerfMode.DoubleRow,
                                            skip_group_check=True,
                                        )
                                else:
                                    nc.tensor.matmul(
                                        ps2[:, c0 : c0 + w],
                                        lhsT=qT8[q_lo:q_hi, pc, :, bass.ts(i, 128)],
                                        rhs=qT8[
                                            q_lo:q_hi,
                                            pc,
                                            :,
                                            jc * 512 + off : (jc + 1) * 512,
                                        ],
                                        start=True,
                                        stop=True,
                                        perf_mode=mybir.MatmulPerfMode.DoubleRow,
                                        skip_group_check=True,
                                    )
                            with nc.allow_low_precision(reason="bf16 probs"):
                                nc.scalar.activation(
                                    pt2[:, 0 : w0 + w1],
                                    ps2[:, 0 : w0 + w1],
                                    mybir.ActivationFunctionType.Exp,
                                    scale=0.125,
                                )
                            flush_h2(h2)
                            pend[h2] = (pc, po2[h2], geom, pt2, gi == len(plist) - 1)
                        # weave filler: next chunk's prep first, then
                        # norm/proj of completed chunks.  No filler on the
                        # chunk's last step: it would sit between the final
                        # S-batch and the boundary flush, delaying the next
                        # chunk's exp stream.
                        if not (pc == NPAIR - 1 and gi == len(plist) - 1):
                            if prep_q:
                                prep_q.popleft()()
                            else:
                                pop_fill(1)
                flush_h2(0)
                flush_h2(1)
                # prep leftovers must finish before the next chunk's S reads qT
                while prep_q:
                    prep_q.popleft()()

                # proj of chunk jc needs all its norms queued/emittable
                for tb in range(4 * jc, 4 * jc + 4):
                    fill_q.append(make_proj_piece(tb, copies_on_act=(jc == NJC - 1)))

            # drain any remaining norm/proj work (chunk 3 + spillover)
            pop_fill(len(fill_q))

    nc.compile()
    return nc


def _build_passthrough():
    """I/O-identical no-compute kernel: isolates transfer+dispatch overhead
    so (wall(kernel) - wall(passthrough)) estimates device compute time."""
    import concourse.bacc as bacc
    import concourse.mybir as mybir

    F32 = mybir.dt.float32
    nc = bacc.Bacc(trn_type="TRN2", target_bir_lowering=False)
    xT_d = nc.dram_tensor("xT", [NF, NS], F32, kind="ExternalInput").ap()
    nc.dram_tensor("wqT", [NF, DL], F32, kind="ExternalInput").ap()
    nc.dram_tensor("bq", [NPAIR, 128], F32, kind="ExternalInput").ap()
    nc.dram_tensor("wpT", [DL, NF], mybir.dt.bfloat16, kind="ExternalInput").ap()
    y_d = nc.dram_tensor("y", [NS, NF], F32, kind="ExternalOutput").ap()
    # bounce same byte volume through SBUF
    import concourse.bass as bass
    import concourse.tile as tile

    xflat = xT_d.rearrange("a b -> (a b)").rearrange("(c x) -> c x", c=2048)
    with tile.TileContext(nc) as tc:
        with tc.tile_pool(name="sb", bufs=2) as sb:
            for i in range(16):
                t = sb.tile([128, 768], F32, tag="t")
                nc.sync.dma_start(t[:], xflat[bass.ts(i, 128), 0:768])
                nc.sync.dma_start(y_d[bass.ts(i, 128), :], t[:])
    nc.compile()
    return nc


def kernel(x, Wqkv_w, Wqkv_b, Wproj_w, Wproj_b, _passthrough=False):
    from concourse.bass_utils import run_bass_kernel_spmd

    x = np.asarray(x, dtype=np.float32)
    Wqkv_w = np.asarray(Wqkv_w, dtype=np.float32)
    Wqkv_b = np.asarray(Wqkv_b, dtype=np.float32)
    Wproj_w = np.asarray(Wproj_w, dtype=np.float32)
    Wproj_b = np.asarray(Wproj_b, dtype=np.float32)

    key = "nc_pt" if _passthrough else "nc"
    if key not in _COMPILED:
        _COMPILED[key] = _build_passthrough() if _passthrough else _build()
    nc = _COMPILED[key]

    import ml_dtypes

    in_maps = []
    for c in range(N_CORES):
        b, g = c // 2, c % 2
        sl = slice(g * DL, (g + 1) * DL)
        in_maps.append(
            {
                "xT": np.ascontiguousarray(x[b].T),
                "wqT": np.ascontiguousarray(Wqkv_w[:NF][sl].T),
                "bq": np.ascontiguousarray(Wqkv_b[:NF][sl].reshape(NPAIR, 128)),
                "wpT": np.ascontiguousarray(Wproj_w[:, sl].T).astype(
                    ml_dtypes.bfloat16
                ),
            }
        )

    trace = bool(int(os.environ.get("KERNEL_TRACE", "0")))
    res = run_bass_kernel_spmd(
        nc,
        in_maps,
        list(range(N_CORES)),
        trace=trace,
        trace_cores=list(range(N_CORES)) if trace else None,
    )
    if trace:
        _COMPILED["exec_time_ns"] = res.exec_time_ns
        _COMPILED["mean_exec_time_ns"] = res.mean_exec_time_ns
        _COMPILED["results_obj"] = res

    y = np.empty((NB, NS, NF), dtype=np.float32)
    for b in range(NB):
        y[b] = res.results[2 * b]["y"] + res.results[2 * b + 1]["y"] + Wproj_b
    return y


# revision 60
# speedup vs baseline: 1.0431x; 1.0116x over previous
"""TRN2 Bass kernel for nn_Attention_73839077752929.

Computes (matching the reference, which has the source bug k = v = q):
    q = x @ Wq^T + bq          (only the q-slice of Wqkv is ever used)
    a = softmax(causal(q q^T / 8)) @ q      per head
    y = a @ Wproj^T + bproj

Sharding: 8 cores = 4 batches x 2 head-groups (6 heads each).
Each core computes a partial projection output for its batch; the host
sums the two partials per batch and adds the projection bias.

On-core scheme (transposed-probability formulation, bf16 attention
operands, f32 PSUM accumulation):
    qT[d,t]   = wqT^T @ xT (+bias)  -> bf16   [head pairs on 128 partitions]
    V_i       = PE-transpose of qT (bf16), ones column pre-set
    PT        = exp((S^T)/8) per k-block PAIR (wide activations,
                multiplicative transposed-causal mask on diag blocks)
    OT'[d|1,q] = sum_i V_i^T @ PT_i    (extra row = softmax denominators)
    aT       *= bcast(1/denominators)   (deferred normalization)
    y[t,o]    = aT^T @ wpT
"""

import os

import numpy as np

N_CORES = 8
NB, NS, NF = 4, 2048, 768
N_HEADS_TOTAL = 12
HD = 64
NH = 6  # heads per core
DL = NH * HD  # 384 local dims
NPAIR = NH // 2  # 3 head pairs (128 partitions each)
NKB = NS // 128  # 16 k-blocks
NJC = NS // 512  # 4 q-chunks
NFC = NF // 128  # 6 feature chunks

_COMPILED = {}


def _build():
    import concourse.bacc as bacc
    import concourse.bass as bass
    import concourse.mybir as mybir
    import concourse.tile as tile
    from concourse.masks import make_identity

    F32 = mybir.dt.float32
    F32R = mybir.dt.float32r
    BF16 = mybir.dt.bfloat16
    F8 = mybir.dt.float8e4

    nc = bacc.Bacc(trn_type="TRN2", target_bir_lowering=False)

    xT_d = nc.dram_tensor("xT", [NF, NS], F32, kind="ExternalInput").ap()
    wqT_d = nc.dram_tensor("wqT", [NF, DL], F32, kind="ExternalInput").ap()
    bq_d = nc.dram_tensor("bq", [NPAIR, 128], F32, kind="ExternalInput").ap()
    wpT_d = nc.dram_tensor("wpT", [DL, NF], BF16, kind="ExternalInput").ap()
    y_d = nc.dram_tensor("y", [NS, NF], F32, kind="ExternalOutput").ap()

    with tile.TileContext(nc) as tc:
        with (
            tc.tile_pool(name="const", bufs=1) as constp,
            tc.tile_pool(name="w", bufs=1) as wp,
            tc.tile_pool(name="big", bufs=1) as bigp,
            tc.tile_pool(name="pt", bufs=9) as ptp,
            tc.tile_pool(name="ys", bufs=4) as ysp,
            tc.tile_pool(name="ps_s", bufs=2, space="PSUM") as ps_s,
            tc.tile_pool(name="ps_o", bufs=2, space="PSUM") as ps_o,
            tc.tile_pool(name="ps_m", bufs=2, space="PSUM") as ps_m,
        ):
            # ---------------- constants ----------------
            identf = constp.tile([128, 128], F32, tag="identf")
            make_identity(nc, identf[:])
            identb = constp.tile([128, 128], BF16, tag="identb")
            # additive transposed-causal bias: 0 where k <= q else -1e10
            # (accumulated into the scores PSUM via an identity matmul BEFORE
            # the score matmul, so exp sees s-1e10 -> 0 with no post-exp op)
            maskBf = constp.tile([128, 128], F32, tag="maskBf")
            nc.gpsimd.memset(maskBf[:], 0.0)
            nc.gpsimd.affine_select(
                out=maskBf[:],
                in_=maskBf[:],
                compare_op=mybir.AluOpType.is_ge,
                fill=-1e10,
                base=0,
                pattern=[[1, 128]],
                channel_multiplier=-1,
            )
            maskB = constp.tile([128, 128], BF16, tag="maskB")
            # e2: [64,128] selector; row0 -> out rows 0:64, row32 -> 64:128
            # (engine partition bases must be multiples of 32, so the two
            # denominator rows live at partitions 0 and 32)
            e2f = constp.tile([64, 128], F32, tag="e2f")
            nc.gpsimd.memset(e2f[:], 0.0)
            nc.gpsimd.memset(e2f[0:1, 0:64], 1.0)
            nc.gpsimd.memset(e2f[32:33, 64:128], 1.0)
            e2 = constp.tile([64, 128], F32R, tag="e2")
            onesf = constp.tile([128, NH], F32, tag="onesf")
            nc.gpsimd.memset(onesf[:], 1.0)
            onesb = constp.tile([128, NH], BF16, tag="onesb")
            zerof = constp.tile([128, 512], F32, tag="zerof")
            nc.gpsimd.memset(zerof[:], 0.0)
            # casts on Act: it is idle during startup, DVE is on the first
            # qT bias-add critical path
            with nc.allow_low_precision(reason="constant casts"):
                nc.scalar.copy(maskB[:], maskBf[:])
                nc.scalar.copy(identb[:], identf[:])
                nc.scalar.copy(e2[:], e2f[:])
                nc.scalar.copy(onesb[:], onesf[:])
            bq_t = constp.tile([128, NPAIR], F32, tag="bq")
            nc.gpsimd.dma_start(bq_t[:], bq_d.rearrange("c p -> p c"))

            # ---------------- weights / activations ----------------
            wqT = wp.tile([128, NFC, DL], F32R, tag="wqT")
            wqT_src = wqT_d.rearrange("(c p) d -> p c d", p=128).bitcast(F32R)
            for pc in range(NPAIR):
                nc.sync.dma_start(
                    wqT[:, :, pc * 128 : (pc + 1) * 128],
                    wqT_src[:, :, pc * 128 : (pc + 1) * 128],
                )
            # wpT is bf16 (host-converted): proj lhsT aT is bf16 and the
            # verifier requires matching dtypes when f32/f32r is involved.
            # On the gpsimd SWDGE ring: keeps both HWDGE queues free for the
            # startup-critical xT/wqT transfers (it's only needed at proj).
            wpT = wp.tile([128, NPAIR, NF], BF16, tag="wpT")
            nc.gpsimd.dma_start(
                wpT[:], wpT_d.rearrange("(c p) o -> p c o", p=128)
            )
            xT = wp.tile([128, NFC, NS], F32R, tag="xT")
            xT_src = xT_d.rearrange("(c p) t -> p c t", p=128).bitcast(F32R)
            # first transfers sized so the first qT matmuls start ASAP; only
            # the startup-critical pieces go on the Act (scalar) queue — later
            # DMA issues on Act would steal engine slots from the exp stream
            nc.scalar.dma_start(xT[:, 0:6, 0:256], xT_src[:, 0:6, 0:256])
            nc.scalar.dma_start(xT[:, :, 256:512], xT_src[:, :, 256:512])
            nc.sync.dma_start(xT[:, :, bass.ts(1, 512)], xT_src[:, :, bass.ts(1, 512)])
            nc.sync.dma_start(xT[:, :, bass.ts(2, 512)], xT_src[:, :, bass.ts(2, 512)])
            nc.sync.dma_start(xT[:, :, bass.ts(3, 512)], xT_src[:, :, bass.ts(3, 512)])

            # ---------------- interleaved: qT / V / attention per 512-chunk ----
            qT = bigp.tile([128, NPAIR, NS], BF16, tag="qT")
            # fp8 copy of qT for the off-diagonal score matmuls, laid out for
            # DoubleRow perf mode: [K=64, 2 k-tiles, cols] per head slice with
            # a ZERO second k-tile plane (out = q0^T k0 + 0^T 0, exact).
            # Numerically validated: off-diagonal scores in e4m3 with bf16
            # diagonal blocks gives 2.7e-3 rel err under faithful fp8
            # emulation (diagonal self-attention scores dominate the error
            # budget and stay bf16).
            qT8 = bigp.tile([128, NPAIR, 2, NS], F8, tag="qT8")
            for pc in range(NPAIR):
                nc.gpsimd.memset(qT8[:, pc, 1, :], 0.0)
            vt = bigp.tile([128, NKB, NH, HD + 1], BF16, tag="vt")
            aT = bigp.tile([128, NPAIR, NS], BF16, tag="aT")
            rs_tiles = []
            for pc in range(NPAIR):
                # [64, NS]: row 32*h2 holds 1/denom for head 2*pc+h2 (all q).
                # Zeroed ONCE here (DMA-wait time): rows other than 0/32 are
                # read by the e2 broadcast matmul multiplied by zero weights,
                # and garbage there could be NaN.
                # zeroed on the Act engine: it is idle during the startup DMA
                # window, while DVE sits on the first qT bias-add critical path
                rs_pc = bigp.tile([64, NS], F32R, tag=f"rs{pc}")
                for jc in range(NJC):
                    with nc.allow_low_precision(reason="f32r zeros"):
                        nc.scalar.copy(rs_pc[:, bass.ts(jc, 512)], zerof[0:64, :])
                rs_tiles.append(rs_pc)

            # k-block pair list per q-chunk jc: [(i0, i1), ...]
            def pairs_for(jc):
                nkb = 4 * jc + 4
                return [(i, i + 1) for i in range(0, nkb, 2)]

            # ---- deferred norm/proj work, woven between attention steps ----
            from collections import deque

            fill_q = deque()

            def pop_fill(n=1):
                for _ in range(min(n, len(fill_q))):
                    fill_q.popleft()()

            def make_norm_piece(jc, pc):
                def piece():
                    prb = ps_m.tile([128, 512], F32, tag="m", name="prb")
                    nc.tensor.matmul(
                        prb[:],
                        lhsT=e2[:],
                        rhs=rs_tiles[pc][:, bass.ts(jc, 512)],
                        start=True,
                        stop=True,
                    )
                    with nc.allow_low_precision(reason="bf16 normalize"):
                        nc.vector.tensor_tensor(
                            aT[:, pc, bass.ts(jc, 512)],
                            aT[:, pc, bass.ts(jc, 512)],
                            prb[:],
                            mybir.AluOpType.mult,
                        )

                return piece

            def make_proj_piece(tb, copies_on_act=False):
                def piece():
                    ysb = ysp.tile([128, NF], F32, tag="y", name="ysb")
                    for o0, on in ((0, 512), (512, 256)):
                        py = ps_m.tile([128, 512], F32, tag="m", name="py")
                        for pc in range(NPAIR):
                            nc.tensor.matmul(
                                py[:, 0:on],
                                lhsT=aT[:, pc, bass.ts(tb, 128)],
                                rhs=wpT[:, pc, o0 : o0 + on],
                                start=(pc == 0),
                                stop=(pc == NPAIR - 1),
                            )
                        if copies_on_act and o0 == 512:
                            # last-chunk pieces drain after the final exp:
                            # Act is idle there, so the small half copies in
                            # parallel with DVE doing the 512-wide half
                            nc.scalar.copy(ysb[:, o0 : o0 + on], py[:, 0:on])
                        else:
                            nc.vector.tensor_copy(ysb[:, o0 : o0 + on], py[:, 0:on])
                    nc.sync.dma_start(y_d[bass.ts(tb, 128), :], ysb[:])

                return piece

            # qT production / V transposes as queueable pieces so chunk
            # jc+1's PE-heavy prep weaves into chunk jc's attention steps
            def make_pq_piece(tck, pc):
                def piece():
                    pq = ps_m.tile([128, 512], F32, tag="m", name="pq")
                    halves = ((0, 256), (256, 256)) if tck == 0 else ((0, 512),)
                    for c0, cn in halves:
                        for fc in range(NFC):
                            nc.tensor.matmul(
                                pq[:, c0 : c0 + cn],
                                lhsT=wqT[:, fc, pc * 128 : (pc + 1) * 128],
                                rhs=xT[:, fc, tck * 512 + c0 : tck * 512 + c0 + cn],
                                start=(fc == 0),
                                stop=(fc == NFC - 1),
                                skip_group_check=True,
                            )
                    with nc.allow_low_precision(reason="bf16 qT"):
                        nc.vector.tensor_scalar_add(
                            qT[:, pc, bass.ts(tck, 512)],
                            pq[:],
                            bq_t[:, pc : pc + 1],
                        )
                    with nc.allow_low_precision(reason="fp8 qT copy"):
                        nc.vector.tensor_scalar_add(
                            qT8[:, pc, 0, bass.ts(tck, 512)],
                            pq[:],
                            bq_t[:, pc : pc + 1],
                        )
                return piece

            def make_v_piece(i):
                # transpose via matmul with a bf16 identity as the MOVING
                # operand (out[k,d] = qT[d,k]; 128 cycles like a bf16
                # transpose, f32 PSUM in the shared "m" slots)
                def piece():
                    for pc in range(NPAIR):
                        pv = ps_m.tile([128, 512], F32, tag="m", name="pv")
                        nc.tensor.matmul(
                            pv[:, 0:128],
                            lhsT=qT[:, pc, bass.ts(i, 128)],
                            rhs=identb[:],
                            start=True,
                            stop=True,
                        )
                        with nc.allow_low_precision(reason="bf16 V"):
                            nc.vector.tensor_copy(
                                vt[:, i, 2 * pc : 2 * pc + 2, 0:HD],
                                pv[:, 0:128].rearrange("k (h d) -> k h d", h=2),
                            )
                    with nc.allow_low_precision(reason="ones col"):
                        nc.vector.tensor_copy(
                            vt[:, i, :, HD : HD + 1],
                            onesb[:].rearrange("p (h u) -> p h u", u=1),
                        )

                return piece

            def qv_pieces(tck):
                return [make_pq_piece(tck, pc) for pc in range(NPAIR)] + [
                    make_v_piece(i) for i in range(4 * tck, 4 * tck + 4)
                ]

            # prologue: only chunk 0's qT pieces run before attention — the
            # first scores need just qT; the V transposes weave in as the
            # first filler pops (V(0),V(1) at step 1, V(2),V(3) at step 2,
            # always ahead of the one-step-delayed AV flushes that read them)
            for pc in range(NPAIR):
                make_pq_piece(0, pc)()

            prep_q = deque()  # qT/V of the NEXT chunk: must drain by chunk end
            prep_q.extend(make_v_piece(i) for i in range(4))
            for tck in range(NJC):
                # ---- attention chunk jc = tck, paired k-blocks.  Emission is
                # software-pipelined for the in-order engine queues: each step
                # emits S+exp for pair g, then the AVs for pair g-1 (whose exp
                # and mask had a full step to complete), plus filler pieces
                # (next chunk's qT/V prep, prior chunks' norm/proj) to keep
                # PE fed and the exp stream continuous across chunks. ----
                jc = tck
                if tck + 1 < NJC:
                    prep_q.extend(qv_pieces(tck + 1))
                pend = {}  # h2 -> (pc, po, geom, pt2, last_of_pc)

                def flush_h2(h2):
                    if h2 not in pend:
                        return
                    f_pc, f_po, f_geom, f_pt, f_last = pend.pop(h2)
                    for i, off, c0, w in f_geom:
                        nc.tensor.matmul(
                            f_po[:, off:512],
                            lhsT=vt[:, i, 2 * f_pc + h2, :],
                            rhs=f_pt[:, c0 : c0 + w],
                            start=(i == 0),
                            stop=(i == 4 * jc + 3),
                            skip_group_check=True,
                        )
                    if f_last:
                        q_lo, q_hi = h2 * HD, (h2 + 1) * HD
                        with nc.allow_low_precision(reason="bf16 aT"):
                            nc.vector.tensor_copy(
                                aT[q_lo:q_hi, f_pc, bass.ts(jc, 512)],
                                f_po[0:HD, :],
                            )
                            nc.vector.reciprocal(
                                rs_tiles[f_pc][32 * h2 : 32 * h2 + 1, bass.ts(jc, 512)],
                                f_po[HD : HD + 1, :],
                            )
                        if h2 == 1:
                            # both heads of this pc reduced: normalize can
                            # weave into the remaining steps right away
                            fill_q.append(make_norm_piece(jc, f_pc))

                for pc in range(NPAIR):
                    po2 = [
                        ps_o.tile([HD + 1, 512], F32, tag="o", name=f"po{h2}")
                        for h2 in range(2)
                    ]
                    plist = pairs_for(jc)
                    for gi, (i0, i1) in enumerate(plist):
                        off0 = max(0, (i0 - 4 * jc) * 128)
                        off1 = max(0, (i1 - 4 * jc) * 128)
                        w0, w1 = 512 - off0, 512 - off1
                        geom = ((i0, off0, 0, w0), (i1, off1, w0, w1))
                        for h2 in (0, 1):
                            q_lo, q_hi = h2 * HD, (h2 + 1) * HD
                            ps2 = ps_s.tile([128, 1024], F32, tag="s")
                            pt2 = ptp.tile([128, 1024], BF16, tag="pt")
                            for i, off, c0, w in geom:
                                if i >= 4 * jc:
                                    # diagonal block: bias its first 128
                                    # stored cols with -1e10 above the
                                    # diagonal, then accumulate bf16 scores
                                    # on top (diagonal self-attention probs
                                    # dominate the error budget: keep bf16)
                                    nc.tensor.matmul(
                                        ps2[:, c0 : c0 + 128],
                                        lhsT=identb[:],
                                        rhs=maskB[:],
                                        start=True,
                                        stop=False,
                                        skip_group_check=True,
                                    )
                                    nc.tensor.matmul(
                                        ps2[:, c0 : c0 + 128],
                                        lhsT=qT[q_lo:q_hi, pc, bass.ts(i, 128)],
                                        rhs=qT[
                                            q_lo:q_hi,
                                            pc,
                                            jc * 512 + off : jc * 512 + off + 128,
                                        ],
                                        start=False,
                                        stop=True,
                                        skip_group_check=True,
                                    )
                                    if w > 128:
                                        nc.tensor.matmul(
                                            ps2[:, c0 + 128 : c0 + w],
                                            lhsT=qT8[q_lo:q_hi, pc, :, bass.ts(i, 128)],
                                            rhs=qT8[
                                                q_lo:q_hi,
                                                pc,
                                                :,
                                                jc * 512 + off + 128 : (jc + 1) * 512,
                                            ],
                                            start=True,
                                            stop=True,
                                            perf_mode=mybir.MatmulPerfMode.DoubleRow,
                                            skip_group_check=True,
                                        )
                                else:
                                    nc.tensor.matmul(
                                        ps2[:, c0 : c0 + w],
                                        lhsT=qT8[q_lo:q_hi, pc, :, bass.ts(i, 128)],
                                        rhs=qT8[
                                            q_lo:q_hi,
                                            pc,
                                            :,
                                            jc * 512 + off : (jc + 1) * 512,
                                        ],
                                        start=True,
                                        stop=True,
                                        perf_mode=mybir.MatmulPerfMode.DoubleRow,
                                        skip_group_check=True,
                                    )
                            with nc.allow_low_precision(reason="bf16 probs"):
                                nc.scalar.activation(
                                    pt2[:, 0 : w0 + w1],
                                    ps2[:, 0 : w0 + w1],
                                    mybir.ActivationFunctionType.Exp,
                                    scale=0.125,
                                )
                            flush_h2(h2)
                            pend[h2] = (pc, po2[h2], geom, pt2, gi == len(plist) - 1)
                        # weave filler: next chunk's prep first, then
                        # norm/proj of completed chunks.  No filler on the
                        # chunk's last step: it would sit between the final
                        # S-batch and the boundary flush, delaying the next
                        # chunk's exp stream.
                        if not (pc == NPAIR - 1 and gi == len(plist) - 1):
                            # chunk 0 is short (6 steps) but must drain its
                            # own V pieces plus chunk 1's prep: pop 2/step
                            for _ in range(2 if jc == 0 else 1):
                                if prep_q:
                                    prep_q.popleft()()
                                else:
                                    pop_fill(1)
                flush_h2(0)
                flush_h2(1)
                # prep leftovers must finish before the next chunk's S reads qT
                while prep_q:
                    prep_q.popleft()()

                # proj of chunk jc needs all its norms queued/emittable
                for tb in range(4 * jc, 4 * jc + 4):
                    fill_q.append(make_proj_piece(tb, copies_on_act=(jc == NJC - 1)))

            # drain any remaining norm/proj work (chunk 3 + spillover)
            pop_fill(len(fill_q))

    nc.compile()
    return nc


def _build_passthrough():
    """I/O-identical no-compute kernel: isolates transfer+dispatch overhead
    so (wall(kernel) - wall(passthrough)) estimates device compute time."""
    import concourse.bacc as bacc
    import concourse.mybir as mybir

    F32 = mybir.dt.float32
    nc = bacc.Bacc(trn_type="TRN2", target_bir_lowering=False)
    xT_d = nc.dram_tensor("xT", [NF, NS], F32, kind="ExternalInput").ap()
    nc.dram_tensor("wqT", [NF, DL], F32, kind="ExternalInput").ap()
    nc.dram_tensor("bq", [NPAIR, 128], F32, kind="ExternalInput").ap()
    nc.dram_tensor("wpT", [DL, NF], mybir.dt.bfloat16, kind="ExternalInput").ap()
    y_d = nc.dram_tensor("y", [NS, NF], F32, kind="ExternalOutput").ap()
    # bounce same byte volume through SBUF
    import concourse.bass as bass
    import concourse.tile as tile

    xflat = xT_d.rearrange("a b -> (a b)").rearrange("(c x) -> c x", c=2048)
    with tile.TileContext(nc) as tc:
        with tc.tile_pool(name="sb", bufs=2) as sb:
            for i in range(16):
                t = sb.tile([128, 768], F32, tag="t")
                nc.sync.dma_start(t[:], xflat[bass.ts(i, 128), 0:768])
                nc.sync.dma_start(y_d[bass.ts(i, 128), :], t[:])
    nc.compile()
    return nc


def kernel(x, Wqkv_w, Wqkv_b, Wproj_w, Wproj_b, _passthrough=False):
    from concourse.bass_utils import run_bass_kernel_spmd

    x = np.asarray(x, dtype=np.float32)
    Wqkv_w = np.asarray(Wqkv_w, dtype=np.float32)
    Wqkv_b = np.asarray(Wqkv_b, dtype=np.float32)
    Wproj_w = np.asarray(Wproj_w, dtype=np.float32)
    Wproj_b = np.asarray(Wproj_b, dtype=np.float32)

    key = "nc_pt" if _passthrough else "nc"
    if key not in _COMPILED:
        _COMPILED[key] = _build_passthrough() if _passthrough else _build()
    nc = _COMPILED[key]

    import ml_dtypes

    in_maps = []
    for c in range(N_CORES):
        b, g = c // 2, c % 2
        sl = slice(g * DL, (g + 1) * DL)
        in_maps.append(
            {
                "xT": np.ascontiguousarray(x[b].T),
                "wqT": np.ascontiguousarray(Wqkv_w[:NF][sl].T),
                "bq": np.ascontiguousarray(Wqkv_b[:NF][sl].reshape(NPAIR, 128)),
                "wpT": np.ascontiguousarray(Wproj_w[:, sl].T).astype(
                    ml_dtypes.bfloat16
                ),
            }
        )

    trace = bool(int(os.environ.get("KERNEL_TRACE", "0")))
    res = run_bass_kernel_spmd(
        nc,
        in_maps,
        list(range(N_CORES)),
        trace=trace,
        trace_cores=list(range(N_CORES)) if trace else None,
    )
    if trace:
        _COMPILED["exec_time_ns"] = res.exec_time_ns
        _COMPILED["mean_exec_time_ns"] = res.mean_exec_time_ns
        _COMPILED["results_obj"] = res

    y = np.empty((NB, NS, NF), dtype=np.float32)
    for b in range(NB):
        y[b] = res.results[2 * b]["y"] + res.results[2 * b + 1]["y"] + Wproj_b
    return y
